# revision 20
# baseline (speedup 1.0000x reference)
"""DecisionTransformer Trainium2 kernel (self-contained).

Sharding: 8 cores = 4 batch pairs x 2 sequence halves. Core r: batch r//2,
half r%2. Half 0 owns timestep blocks {0,3} (of 4 x 128-ts blocks), half 1
owns {1,2} -- balances causal-attention work exactly. Per layer each pair
AllGathers its K/V (bf16, one collective); everything else is token-local
with full weights.

Activations live transposed ([H, tokens]) so matmuls chain with zero
transposes. LayerNorm stats via PE ones-matmul column sums. Softmax skips
max-subtraction (scores are small for this model; verified vs reference).
att@V carries a ones-column per head so softmax denominators fall out of
the same matmul. bf16 matmuls, f32 PSUM accumulation, f32 residual stream.
One SPMD program for all cores; the causal-structure difference between
sequence halves is encoded in per-core mask input data.
"""
import numpy as np
import ml_dtypes

import concourse.bacc as bacc
import concourse.bass as bass
import concourse.tile as tile
from concourse import mybir

bf16 = ml_dtypes.bfloat16

B, T = 4, 512
STATE_DIM, ACT_DIM = 17, 18
H, NH, L, MAXT = 1024, 16, 4, 4096
S = 3 * T
D = H // NH
FF = 4 * H
LN_EPS = 1e-5
HP = H // 128
FFP = FF // 128

GCHUNK = [[0, 3], [1, 2]]

f32 = mybir.dt.float32
bf = mybir.dt.bfloat16
i32 = mybir.dt.int32
AF = mybir.ActivationFunctionType


def _strided(ap, start, step, count):
    return bass.AP(tensor=ap.tensor, offset=ap.offset + start,
                   ap=[list(ap.ap[0]), [step, count]])


def build_program(n_cores, own_chunks_list, uniform, n_layers=None,
                  skip_ag=False, skip_attn=False, skip_mlp=False, skip_qkv=False,
                  skip_coll=False):
    SLOTS = len(own_chunks_list)
    N_TS = 128 * SLOTS
    NT = 3 * N_TS
    NKT = 12 if uniform else 3 * (max(own_chunks_list) + 1)
    slot_valid, slot_masked, masked_pairs = [], [], []
    for j, g in enumerate(own_chunks_list):
        if uniform:
            valid = list(range(6)) if j == 0 else list(range(12))
            masked = list(range(6)) if j == 0 else list(range(6, 12))
        else:
            valid = list(range(3 * (g + 1)))
            masked = valid[-3:]
        slot_valid.append(valid)
        mm_ = {}
        for t in masked:
            mm_[t] = len(masked_pairs)
            masked_pairs.append((j, t))
        slot_masked.append(mm_)
    NMASK = len(masked_pairs)

    nc = bacc.Bacc("TRN2", target_bir_lowering=False, debug=False,
                   num_devices=n_cores)

    ts_idx = nc.dram_tensor("ts_idx", [128, SLOTS], i32, kind="ExternalInput")
    act_idx = nc.dram_tensor("act_idx", [128, SLOTS], i32, kind="ExternalInput")
    states_T = nc.dram_tensor("states_T", [STATE_DIM + 1, N_TS], f32, kind="ExternalInput")
    rtg_T = nc.dram_tensor("rtg_T", [2, N_TS], f32, kind="ExternalInput")
    masks_in = nc.dram_tensor("masks_in", [128, NMASK, 384], bf, kind="ExternalInput")
    ttab = nc.dram_tensor("ttab", [MAXT, H], f32, kind="ExternalInput")
    atab = nc.dram_tensor("atab", [ACT_DIM, H], f32, kind="ExternalInput")
    wst_in = nc.dram_tensor("wst_in", [STATE_DIM + 1, H], f32, kind="ExternalInput")
    wrtg_in = nc.dram_tensor("wrtg_in", [2, H], f32, kind="ExternalInput")
    ident_in = nc.dram_tensor("ident_in", [128, 128], f32, kind="ExternalInput")
    watt = nc.dram_tensor("watt", [L, 4, H, H], bf, kind="ExternalInput")
    w1_in = nc.dram_tensor("w1_in", [L, H, FF], bf, kind="ExternalInput")
    w2_in = nc.dram_tensor("w2_in", [L, FF, H], bf, kind="ExternalInput")
    ba_in = nc.dram_tensor("ba_in", [128, L * 4, HP], f32, kind="ExternalInput")
    b1_in = nc.dram_tensor("b1_in", [128, L, FFP], f32, kind="ExternalInput")
    b2_in = nc.dram_tensor("b2_in", [128, L, HP], f32, kind="ExternalInput")
    bv_bc = nc.dram_tensor("bv_bc", [L, 2, 512], f32, kind="ExternalInput")
    wh_in = nc.dram_tensor("wh_in", [H, 36], bf, kind="ExternalInput")
    bh_in = nc.dram_tensor("bh_in", [36], f32, kind="ExternalInput")

    spT_o = nc.dram_tensor("spT_o", [STATE_DIM, N_TS], f32, kind="ExternalOutput")
    apT_o = nc.dram_tensor("apT_o", [ACT_DIM, N_TS], f32, kind="ExternalOutput")
    rpT_o = nc.dram_tensor("rpT_o", [1, N_TS], f32, kind="ExternalOutput")

    KV_N = H * NT
    if uniform:
        cc_in_t = nc.dram_tensor("cc_in", [2 * KV_N], bf)
        cc_out_t = nc.dram_tensor("cc_out", [4 * KV_N], bf)

    NL = L if n_layers is None else n_layers
    with tile.TileContext(nc) as tc:
        with tc.tile_pool(name="sb", bufs=1) as sbp, \
             tc.tile_pool(name="rot", bufs=3) as rot, \
             tc.tile_pool(name="pp", bufs=8, space="PSUM") as pp:

            def P():
                return pp.tile([128, 512], f32, tag="mm", name="psmm")

            hT = sbp.tile([128, HP, NT], f32, tag="hT")
            h_bf = sbp.tile([128, HP, NT], bf, tag="h_bf")
            oT = sbp.tile([128, HP, NT], bf, tag="oT")
            qT = sbp.tile([128, HP, NT], bf, tag="qT")
            kT_all = sbp.tile([128, HP, 128 * NKT], bf, tag="kT_all")
            v_all = sbp.tile([128, NKT, 16 * 65], bf, tag="v_all")
            masks = sbp.tile([128, NMASK, 384], bf, tag="masks")
            ba_t = sbp.tile([128, L * 4, HP], f32, tag="ba_t")
            b1_t = sbp.tile([128, L, FFP], f32, tag="b1_t")
            b2_t = sbp.tile([128, L, HP], f32, tag="b2_t")
            bh_t = sbp.tile([36, 1], f32, tag="bh_t")
            ones_f = sbp.tile([128, 1], f32, tag="ones_f")
            ones_r = sbp.tile([1, 128], f32, tag="ones_r")
            ones_b = sbp.tile([128, 1], bf, tag="ones_b")
            eps_t = sbp.tile([1, 1], f32, tag="eps_t")
            wh_t = sbp.tile([128, HP, 36], bf, tag="wh_t")

            nc.gpsimd.dma_start(masks[:], masks_in[:])
            nc.gpsimd.dma_start(ba_t[:], ba_in[:])
            nc.gpsimd.dma_start(b1_t[:], b1_in[:])
            nc.gpsimd.dma_start(b2_t[:], b2_in[:])
            nc.gpsimd.dma_start(bh_t[:], bass.AP(tensor=bh_in, offset=0, ap=[[1, 36], [0, 1]]))
            nc.gpsimd.dma_start(wh_t[:], wh_in[:].rearrange("(kt p) o -> p kt o", p=128))
            nc.vector.memset(ones_f[:], 1.0)
            nc.vector.memset(ones_r[:], 1.0)
            nc.vector.memset(ones_b[:], 1.0)
            nc.vector.memset(eps_t[:], LN_EPS)
            for h in range(16):
                nc.vector.memset(v_all[:, :, 65 * h + 64], 1.0)

            # ---------- P0: embeddings ----------
            with tc.tile_pool(name="ep", bufs=1) as ep:
                ident = ep.tile([128, 128], f32, tag="ident")
                sb_ts = ep.tile([128, SLOTS], i32, tag="sb_ts")
                sb_ai = ep.tile([128, SLOTS], i32, tag="sb_ai")
                sb_st = ep.tile([STATE_DIM + 1, N_TS], f32, tag="sb_st")
                sb_rt = ep.tile([2, N_TS], f32, tag="sb_rt")
                wst_t = ep.tile([STATE_DIM + 1, H], f32, tag="wst_t")
                wrtg_t = ep.tile([2, H], f32, tag="wrtg_t")
                nc.gpsimd.dma_start(ident[:], ident_in[:])
                nc.gpsimd.dma_start(sb_ts[:], ts_idx[:])
                nc.gpsimd.dma_start(sb_ai[:], act_idx[:])
                nc.gpsimd.dma_start(sb_st[:], states_T[:])
                nc.gpsimd.dma_start(sb_rt[:], rtg_T[:])
                nc.gpsimd.dma_start(wst_t[:], wst_in[:])
                nc.gpsimd.dma_start(wrtg_t[:], wrtg_in[:])

                for u2 in range(SLOTS):
                    te = ep.tile([128, H], f32, tag="te", bufs=1)
                    nc.gpsimd.indirect_dma_start(
                        out=te[:], out_offset=None, in_=ttab[:],
                        in_offset=bass.IndirectOffsetOnAxis(ap=sb_ts[:, u2:u2 + 1], axis=0))
                    ae = ep.tile([128, H], f32, tag="ae", bufs=1)
                    nc.gpsimd.indirect_dma_start(
                        out=ae[:], out_offset=None, in_=atab[:],
                        in_offset=bass.IndirectOffsetOnAxis(ap=sb_ai[:, u2:u2 + 1], axis=0))
                    for ch in range(2):
                        cs = slice(512 * ch, 512 * ch + 512)
                        nat = [ep.tile([128, 512], f32, tag=f"nat{ty}",
                                       name=f"nat{ty}", bufs=2) for ty in range(3)]
                        nc.vector.tensor_add(nat[2][:], ae[:, cs], te[:, cs])
                        ps = P()
                        nc.tensor.matmul(ps[:], sb_st[:, 128 * u2:128 * u2 + 128],
                                         wst_t[:, cs], start=True, stop=True)
                        nc.vector.tensor_add(nat[1][:], ps[:], te[:, cs])
                        pr_ = P()
                        nc.tensor.matmul(pr_[:], sb_rt[:, 128 * u2:128 * u2 + 128],
                                         wrtg_t[:, cs], start=True, stop=True)
                        nc.vector.tensor_add(nat[0][:], pr_[:], te[:, cs])
                        for ty in range(3):
                            for k4 in range(4):
                                kt = 4 * ch + k4
                                pt = P()
                                nc.tensor.transpose(pt[:, 0:128],
                                                    nat[ty][:, 128 * k4:128 * k4 + 128],
                                                    ident[:])
                                dst = _strided(hT[:, kt, :], 384 * u2 + ty, 3, 128)
                                nc.vector.tensor_copy(dst, pt[:, 0:128])

            # ---------- LN (in-place on hT, refresh h_bf) ----------
            def ln_inplace():
                for c in range(SLOTS):
                    cs = slice(384 * c, 384 * c + 384)
                    ab_t = rot.tile([1, 2, 384], f32, tag="ab_t", name="ab_t", bufs=3)
                    ps1 = P()
                    ps2 = P()
                    for kt in range(HP):
                        sq = rot.tile([128, 384], bf, tag="sq", name="sq")
                        nc.scalar.activation(sq[:], hT[:, kt, cs], AF.Square)
                        nc.tensor.matmul(ps1[0:1, 0:384], ones_f[:], hT[:, kt, cs],
                                         start=(kt == 0), stop=(kt == HP - 1))
                        nc.tensor.matmul(ps2[0:1, 0:384], ones_b[:], sq[:],
                                         start=(kt == 0), stop=(kt == HP - 1))
                    mean = rot.tile([1, 384], f32, tag="mean")
                    e2 = rot.tile([1, 384], f32, tag="e2")
                    nc.scalar.activation(mean[:], ps1[0:1, 0:384], AF.Copy, scale=1.0 / H)
                    nc.scalar.activation(e2[:], ps2[0:1, 0:384], AF.Copy, scale=1.0 / H)
                    var = rot.tile([1, 384], f32, tag="var")
                    nc.vector.tensor_mul(var[:], mean[:], mean[:])
                    nc.vector.tensor_tensor(out=var[:], in0=e2[:], in1=var[:],
                                            op=mybir.AluOpType.subtract)
                    nc.scalar.activation(var[:], var[:], AF.Sqrt, bias=eps_t[:], scale=1.0)
                    nc.vector.reciprocal(ab_t[:, 0, :], var[:])
                    nc.vector.tensor_mul(ab_t[:, 1, :], mean[:], ab_t[:, 0, :])
                    nc.scalar.mul(ab_t[:, 1, :], ab_t[:, 1, :], -1.0)
                    pa = P()
                    nc.tensor.matmul(pa[:, 0:384], ones_r[:], ab_t[:, 0, :],
                                     start=True, stop=True)
                    pb = P()
                    nc.tensor.matmul(pb[:, 0:384], ones_r[:], ab_t[:, 1, :],
                                     start=True, stop=True)
                    for kt in range(HP):
                        nc.vector.tensor_mul(hT[:, kt, cs], hT[:, kt, cs], pa[:, 0:384])
                        nc.vector.tensor_add(hT[:, kt, cs], hT[:, kt, cs], pb[:, 0:384])
                        nc.vector.tensor_copy(h_bf[:, kt, cs], hT[:, kt, cs])

            ln_inplace()

            # ---------- layers ----------
            with tc.tile_pool(name="wp", bufs=4) as wp:

                def load_half(src_ap, n_mid):
                    w = wp.tile([128, n_mid, 8 * 1024 // n_mid // 2], bf,
                                tag="w", name="wtile")
                    nc.gpsimd.dma_start(w[:], src_ap)
                    return w

                def load_att_w(l, j, half):
                    # [128, HP, 512]: out-cols half*512 ... half*512+512
                    return load_half(
                        watt[l, j][:, 512 * half:512 * half + 512].rearrange(
                            "(kt p) o -> p kt o", p=128), HP)

                for l_iter in range(NL):
                    l = l_iter % L
                    # ---- QKV ----
                    for (is_q, bj) in (() if skip_qkv else ((True, 0), (False, 1))):
                        for half in range(2):
                            wt = load_att_w(l, bj, half)
                            for o4 in range(4):
                                ot = 4 * half + o4
                                for c in range(SLOTS):
                                    cs = slice(384 * c, 384 * c + 384)
                                    ps = P()
                                    for kt in range(HP):
                                        nc.tensor.matmul(
                                            ps[:, 0:384], wt[:, kt, 128 * o4:128 * o4 + 128],
                                            h_bf[:, kt, cs],
                                            start=(kt == 0), stop=(kt == HP - 1))
                                    if is_q:
                                        nc.scalar.activation(
                                            qT[:, ot, cs], ps[:, 0:384], AF.Identity,
                                            bias=ba_t[:, 4 * l + bj, ot:ot + 1])
                                    else:
                                        g = own_chunks_list[c]
                                        nc.scalar.activation(
                                            kT_all[:, ot, 384 * g:384 * g + 384],
                                            ps[:, 0:384], AF.Identity,
                                            bias=ba_t[:, 4 * l + bj, ot:ot + 1])
                    for dc in (() if skip_qkv else range(2)):
                        wv = load_att_w(l, 2, dc)
                        bvb = rot.tile([128, 512], f32, tag="tmp", name="bvb", bufs=2)
                        nc.gpsimd.dma_start(
                            bvb[:], bass.AP(tensor=bv_bc, offset=(l * 2 + dc) * 512,
                                            ap=[[0, 128], [1, 512]]))
                        for tt in range(NT // 128):
                            g = own_chunks_list[tt // 3]
                            gt = 3 * g + (tt % 3)
                            ps = P()
                            for kt in range(HP):
                                nc.tensor.matmul(ps[:], h_bf[:, kt, 128 * tt:128 * tt + 128],
                                                 wv[:, kt, :],
                                                 start=(kt == 0), stop=(kt == HP - 1))
                            dst = bass.AP(
                                tensor=v_all.tensor,
                                offset=v_all[:].offset + gt * (16 * 65) + (8 * dc) * 65,
                                ap=[list(v_all[:].ap[0]), [65, 8], [1, 64]])
                            nc.vector.tensor_tensor(
                                out=dst, in0=ps[:].rearrange("p (h d) -> p h d", h=8),
                                in1=bvb[:].rearrange("p (h d) -> p h d", h=8),
                                op=mybir.AluOpType.add)

                    # ---- AllGather K/V ----
                    if uniform and not skip_ag:
                        cc_in = cc_in_t.ap()
                        cc_out = cc_out_t.ap()
                        k_sec = cc_in[0:KV_N].rearrange("(r c) -> r c", c=NT)
                        v_sec = cc_in[KV_N:2 * KV_N].rearrange("(r c) -> r c", c=H)
                        for c in range(SLOTS):
                            g = own_chunks_list[c]
                            nc.gpsimd.dma_start(
                                k_sec[:, 384 * c:384 * c + 384].rearrange(
                                    "(kt p) q -> p kt q", p=128),
                                kT_all[:, :, 384 * g:384 * g + 384])
                            for tt in range(3):
                                src = bass.AP(
                                    tensor=v_all.tensor,
                                    offset=v_all[:].offset + (3 * g + tt) * (16 * 65),
                                    ap=[list(v_all[:].ap[0]), [65, 16], [1, 64]])
                                nc.sync.dma_start(
                                    v_sec[384 * c + 128 * tt:384 * c + 128 * tt + 128,
                                          :].rearrange("p (h d) -> p h d", h=16), src)
                        if not skip_coll:
                            groups = [[2 * i, 2 * i + 1] for i in range(n_cores // 2)]
                            nc.gpsimd.collective_compute(
                                "AllGather", mybir.AluOpType.bypass, replica_groups=groups,
                                ins=[cc_in], outs=[cc_out])
                        else:
                            nc.sync.dma_start(cc_out[0:2 * KV_N], cc_in[:])
                        for rr in range(2):
                            sec = cc_out[rr * 2 * KV_N: (rr + 1) * 2 * KV_N]
                            k_s = sec[0:KV_N].rearrange("(r c) -> r c", c=NT)
                            v_s = sec[KV_N:2 * KV_N].rearrange("(r c) -> r c", c=H)
                            for c in range(SLOTS):
                                g = GCHUNK[rr][c]
                                nc.gpsimd.dma_start(
                                    kT_all[:, :, 384 * g:384 * g + 384],
                                    k_s[:, 384 * c:384 * c + 384].rearrange(
                                        "(kt p) q -> p kt q", p=128))
                                for tt in range(3):
                                    dstv = bass.AP(
                                        tensor=v_all.tensor,
                                        offset=v_all[:].offset + (3 * g + tt) * (16 * 65),
                                        ap=[list(v_all[:].ap[0]), [65, 16], [1, 64]])
                                    nc.sync.dma_start(
                                        dstv,
                                        v_s[384 * c + 128 * tt:384 * c + 128 * tt + 128,
                                            :].rearrange("p (h d) -> p h d", h=16))

                    # ---- attention ----
                    if skip_attn:
                        nc.vector.memset(oT[:], 0.0)
                    for hp_ in (() if skip_attn else range(8)):
                        for j in range(SLOTS):
                            qs = slice(384 * j, 384 * j + 384)
                            po = [P(), P()]
                            valid = slot_valid[j]
                            for ti, t in enumerate(valid):
                                for hh in range(2):
                                    h = 2 * hp_ + hh
                                    bp = 64 * hh
                                    ps = P()
                                    nc.tensor.matmul(
                                        ps[:, 0:384],
                                        kT_all[bp:bp + 64, hp_, 128 * t:128 * t + 128],
                                        qT[bp:bp + 64, hp_, qs], start=True, stop=True)
                                    es = rot.tile([128, 384], bf, tag="es", name="es", bufs=3)
                                    nc.scalar.activation(es[:], ps[:, 0:384], AF.Exp,
                                                         scale=0.125)
                                    mi = slot_masked[j].get(t)
                                    if mi is not None:
                                        nc.gpsimd.tensor_mul(es[:], es[:], masks[:, mi, :])
                                    nc.tensor.matmul(
                                        po[hh][0:65, 0:384],
                                        v_all[:, t, 65 * h:65 * h + 65], es[:],
                                        start=(ti == 0), stop=(ti == len(valid) - 1))
                            rcp = rot.tile([1, 2, 384], f32, tag="rcp", name="rcp", bufs=4)
                            for hh in range(2):
                                nc.vector.reciprocal(rcp[:, hh, :], po[hh][64:65, 0:384])
                            for hh in range(2):
                                pr = P()
                                nc.tensor.matmul(pr[0:64, 0:384], ones_r[:, 0:64],
                                                 rcp[:, hh, :], start=True, stop=True)
                                prs = rot.tile([64, 384], f32, tag="prs", name="prs",
                                               bufs=2)
                                nc.scalar.activation(prs[:], pr[0:64, 0:384], AF.Copy)
                                osl = oT[64 * hh:64 * hh + 64, hp_, qs]
                                nc.vector.tensor_mul(osl, po[hh][0:64, 0:384], prs[:])

                    # ---- proj + residual + LN ----
                    for half in range(2):
                        wo = load_att_w(l, 3, half)
                        for o4 in range(4):
                            ot = 4 * half + o4
                            for c in range(SLOTS):
                                cs = slice(384 * c, 384 * c + 384)
                                ps = P()
                                for kt in range(HP):
                                    nc.tensor.matmul(
                                        ps[:, 0:384], wo[:, kt, 128 * o4:128 * o4 + 128],
                                        oT[:, kt, cs], start=(kt == 0), stop=(kt == HP - 1))
                                tmp = rot.tile([128, 512], f32, tag="tmp", name="tmp", bufs=2)
                                nc.scalar.activation(tmp[:, 0:384], ps[:, 0:384], AF.Identity,
                                                     bias=ba_t[:, 4 * l + 3, ot:ot + 1])
                                nc.vector.tensor_add(hT[:, ot, cs], hT[:, ot, cs],
                                                     tmp[:, 0:384])
                    ln_inplace()

                    # ---- MLP (accumulate groups directly into hT) ----
                    for grp in (() if skip_mlp else range(4)):
                        w1a = load_half(
                            w1_in[l][:, 1024 * grp:1024 * grp + 512].rearrange(
                                "(kt p) o -> p kt o", p=128), HP)
                        w1b = load_half(
                            w1_in[l][:, 1024 * grp + 512:1024 * grp + 1024].rearrange(
                                "(kt p) o -> p kt o", p=128), HP)
                        w2a = load_half(
                            w2_in[l][1024 * grp:1024 * grp + 512, :].rearrange(
                                "(ft p) o -> p ft o", p=128), 4)
                        w2b = load_half(
                            w2_in[l][1024 * grp + 512:1024 * grp + 1024, :].rearrange(
                                "(ft p) o -> p ft o", p=128), 4)
                        for c in range(NT // 256):
                            cs = slice(256 * c, 256 * c + 256)
                            mst = rot.tile([128, 8, 256], bf, tag="mst", name="mst", bufs=2)
                            for ft in range(8):
                                w1h = w1a if ft < 4 else w1b
                                f4 = ft % 4
                                pm = P()
                                for kt in range(HP):
                                    nc.tensor.matmul(
                                        pm[:, 0:256], w1h[:, kt, 128 * f4:128 * f4 + 128],
                                        h_bf[:, kt, cs],
                                        start=(kt == 0), stop=(kt == HP - 1))
                                nc.scalar.activation(
                                    mst[:, ft, :], pm[:, 0:256], AF.Gelu,
                                    bias=b1_t[:, l, 8 * grp + ft:8 * grp + ft + 1])
                            for oh in range(2):
                                ph = [P() for _ in range(4)]
                                for ft in range(8):
                                    w2h = w2a if ft < 4 else w2b
                                    f4 = ft % 4
                                    for o4 in range(4):
                                        ot = 4 * oh + o4
                                        nc.tensor.matmul(
                                            ph[o4][:, 0:256],
                                            w2h[:, f4, 128 * ot:128 * ot + 128],
                                            mst[:, ft, :], start=(ft == 0), stop=(ft == 7))
                                for o4 in range(4):
                                    ot = 4 * oh + o4
                                    nc.vector.tensor_add(hT[:, ot, cs], hT[:, ot, cs],
                                                         ph[o4][:, 0:256])
                    # b2 bias
                    for ot in range(HP):
                        nc.vector.tensor_scalar(
                            out=hT[:, ot, :], in0=hT[:, ot, :],
                            scalar1=b2_t[:, l, ot:ot + 1], scalar2=None,
                            op0=mybir.AluOpType.add)
                    ln_inplace()

                # ---------- final heads ----------
                for c in range(SLOTS):
                    ps = P()
                    for kt in range(HP):
                        rhs_a = _strided(h_bf[:, kt, :], 384 * c + 2, 3, 128)
                        nc.tensor.matmul(ps[0:36, 0:128], wh_t[:, kt, :], rhs_a,
                                         start=(kt == 0), stop=(kt == HP - 1))
                    out_s = rot.tile([36, 128], f32, tag="out_s", name="out_s")
                    nc.scalar.activation(out_s[:], ps[0:36, 0:128], AF.Identity, bias=bh_t[:])
                    nc.gpsimd.dma_start(spT_o[:, 128 * c:128 * c + 128], out_s[0:17, :])
                    nc.gpsimd.dma_start(rpT_o[:, 128 * c:128 * c + 128], out_s[35:36, :])
                    ps2 = P()
                    for kt in range(HP):
                        rhs_s = _strided(h_bf[:, kt, :], 384 * c + 1, 3, 128)
                        nc.tensor.matmul(ps2[0:36, 0:128], wh_t[:, kt, :], rhs_s,
                                         start=(kt == 0), stop=(kt == HP - 1))
                    out_s2 = rot.tile([36, 128], f32, tag="out_s", name="out_s2")
                    nc.scalar.activation(out_s2[:], ps2[0:36, 0:128], AF.Identity,
                                         bias=bh_t[:])
                    nc.gpsimd.dma_start(apT_o[:, 128 * c:128 * c + 128], out_s2[17:35, :])

    nc.compile()
    meta = dict(SLOTS=SLOTS, N_TS=N_TS, NT=NT, NKT=NKT,
                masked_pairs=masked_pairs, own_chunks_list=own_chunks_list)
    return nc, meta


# ====================== host side ======================

def make_masks(meta):
    oc = meta["own_chunks_list"]
    NM = len(meta["masked_pairs"])
    m = np.zeros((128, NM, 384), np.float32)
    for i, (j, t) in enumerate(meta["masked_pairs"]):
        g = oc[j]
        kpos = 128 * t + np.arange(128)[:, None]
        qpos = 384 * g + np.arange(384)[None, :]
        m[:, i, :] = (kpos <= qpos).astype(np.float32)
    return m.astype(bf16)


def make_core_inputs(inputs, half, b, meta, shared):
    chunks = meta["own_chunks_list"]
    ts_sl = np.concatenate([np.arange(128 * g, 128 * g + 128) for g in chunks])
    d = dict(shared)
    d["ts_idx"] = np.asarray(inputs["timesteps"])[b, ts_sl].astype(np.int32).reshape(
        len(chunks), 128).T.copy()
    d["act_idx"] = np.asarray(inputs["actions"])[b, ts_sl].astype(np.int32).reshape(
        len(chunks), 128).T.copy()
    st = np.asarray(inputs["states"], np.float32)[b, ts_sl].T  # [17, N_TS]
    d["states_T"] = np.concatenate([st, np.ones((1, st.shape[1]), np.float32)])
    rt = np.asarray(inputs["returns_to_go"], np.float32)[b, ts_sl].T  # [1, N_TS]
    d["rtg_T"] = np.concatenate([rt, np.ones((1, rt.shape[1]), np.float32)])
    return d


class SpmdRunner:
    def __init__(self, nc, n_cores):
        import jax
        from jax.sharding import Mesh, PartitionSpec
        from jax.experimental.shard_map import shard_map
        from concourse.bass2jax import (_bass_exec_p, install_neuronx_cc_hook,
                                        partition_id_tensor)
        import concourse.mybir as mybir_
        self.jax = jax
        install_neuronx_cc_hook()
        self.nc = nc
        self.n_cores = n_cores
        partition_name = nc.partition_id_tensor.name if nc.partition_id_tensor else None
        in_names, out_names, out_avals = [], [], []
        for alloc in nc.m.functions[0].allocations:
            if not isinstance(alloc, mybir_.MemoryLocationSet):
                continue
            name = alloc.memorylocations[0].name
            if alloc.kind == "ExternalInput":
                if name != partition_name:
                    in_names.append(name)
            elif alloc.kind == "ExternalOutput":
                out_names.append(name)
                out_avals.append(jax.core.ShapedArray(
                    tuple(alloc.tensor_shape), mybir_.dt.np(alloc.dtype)))
        self.in_names, self.out_names, self.out_avals = in_names, out_names, out_avals
        n_params = len(in_names)
        self._zero_outs = [np.zeros(a.shape, a.dtype) for a in out_avals]
        all_in = list(in_names) + list(out_names)
        if partition_name is not None:
            all_in.append(partition_name)

        def _body(*args):
            operands = list(args)
            if partition_name is not None:
                operands.append(partition_id_tensor())
            outs = _bass_exec_p.bind(
                *operands, out_avals=tuple(out_avals), in_names=tuple(all_in),
                out_names=tuple(out_names), lowering_input_output_aliases=(),
                sim_require_finite=False, sim_require_nnan=False, nc=nc)
            return tuple(outs)

        devices = jax.devices()[:n_cores]
        self.mesh = Mesh(np.asarray(devices), ("core",))
        in_specs = (PartitionSpec("core"),) * (n_params + len(out_names))
        out_specs = (PartitionSpec("core"),) * len(out_names)
        self._fn = jax.jit(shard_map(_body, mesh=self.mesh, in_specs=in_specs,
                                     out_specs=out_specs, check_rep=False))

    def stage_inputs(self, in_maps):
        import jax
        from jax.sharding import PartitionSpec
        n = self.n_cores
        per_core = [[np.asarray(m[name]) for name in self.in_names] for m in in_maps]
        concat = [np.concatenate([per_core[c][i] for c in range(n)], axis=0)
                  for i in range(len(self.in_names))]
        concat += [np.zeros((n * z.shape[0], *z.shape[1:]), z.dtype)
                   for z in self._zero_outs]
        sharding = jax.sharding.NamedSharding(self.mesh, PartitionSpec("core"))
        return [jax.device_put(c, sharding) for c in concat]

    def run_staged(self, staged):
        out = self._fn(*staged)
        self.jax.block_until_ready(out)
        return out

    def results(self, out_arrs):
        n = self.n_cores
        return [{name: np.asarray(out_arrs[i]).reshape(n, *self.out_avals[i].shape)[c]
                 for i, name in enumerate(self.out_names)} for c in range(n)]

    def run(self, in_maps):
        return self.results(self.run_staged(self.stage_inputs(in_maps)))


def _shared_inputs(inputs, meta):
    sh = {}
    sh["masks_in"] = make_masks(meta)
    sh["ttab"] = np.asarray(inputs["embed_time_table"], np.float32)
    sh["atab"] = np.asarray(inputs["embed_act_table"], np.float32)
    sh["wst_in"] = np.concatenate([np.asarray(inputs["Wst"], np.float32),
                                   np.asarray(inputs["bst"], np.float32)[None, :]])
    sh["wrtg_in"] = np.concatenate([np.asarray(inputs["Wrtg"], np.float32),
                                    np.asarray(inputs["brtg"], np.float32)[None, :]])
    sh["ident_in"] = np.eye(128, dtype=np.float32)
    watt = np.stack([np.asarray(inputs[k], np.float32) for k in
                     ("Wq", "Wk", "Wv", "Wo")], axis=1)
    sh["watt"] = watt.astype(bf16)
    sh["w1_in"] = np.asarray(inputs["W1"], np.float32).astype(bf16)
    sh["w2_in"] = np.asarray(inputs["W2"], np.float32).astype(bf16)
    ba = np.stack([np.asarray(inputs[k], np.float32) for k in
                   ("bq", "bk", "bv", "bo")], axis=1)          # [L,4,H]
    sh["ba_in"] = np.ascontiguousarray(
        ba.reshape(L, 4, HP, 128).transpose(3, 0, 1, 2).reshape(128, L * 4, HP))
    b1 = np.asarray(inputs["b1"], np.float32)                   # [L,FF]
    sh["b1_in"] = np.ascontiguousarray(
        b1.reshape(L, FFP, 128).transpose(2, 0, 1))
    b2 = np.asarray(inputs["b2"], np.float32)                   # [L,H]
    sh["b2_in"] = np.ascontiguousarray(
        b2.reshape(L, HP, 128).transpose(2, 0, 1))
    sh["bv_bc"] = np.ascontiguousarray(ba[:, 2, :].reshape(L, 2, 512))
    wh = np.concatenate([np.asarray(inputs["Wps"], np.float32),
                         np.asarray(inputs["Wpa"], np.float32),
                         np.asarray(inputs["Wpr"], np.float32)], axis=1)
    sh["wh_in"] = wh.astype(bf16)
    sh["bh_in"] = np.concatenate([np.asarray(inputs["bps"], np.float32),
                                  np.asarray(inputs["bpa"], np.float32),
                                  np.asarray(inputs["bpr"], np.float32)])
    return sh


_CACHE = {}


def kernel(**inputs):
    if "runner" not in _CACHE:
        nc, meta0 = build_program(8, GCHUNK[0], uniform=True)
        _CACHE["prog"] = (nc, meta0)
        _CACHE["runner"] = SpmdRunner(nc, 8)
        _CACHE["meta_by_half"] = [dict(meta0, own_chunks_list=GCHUNK[0]),
                                  dict(meta0, own_chunks_list=GCHUNK[1])]
    runner = _CACHE["runner"]
    meta_by_half = _CACHE["meta_by_half"]
    shared = [_shared_inputs(inputs, meta_by_half[0])]
    sh1 = dict(shared[0])
    sh1["masks_in"] = make_masks(meta_by_half[1])
    shared.append(sh1)
    in_maps = [make_core_inputs(inputs, r % 2, r // 2, meta_by_half[r % 2],
                                shared[r % 2]) for r in range(8)]
    staged = runner.stage_inputs(in_maps)
    _CACHE["last_staged"] = staged
    res = runner.results(runner.run_staged(staged))

    state_preds = np.zeros((B, T, STATE_DIM), np.float32)
    action_preds = np.zeros((B, T, ACT_DIM), np.float32)
    return_preds = np.zeros((B, T, 1), np.float32)
    for r in range(8):
        b, half = r // 2, r % 2
        for c, g in enumerate(GCHUNK[half]):
            sl = slice(128 * g, 128 * g + 128)
            cc = slice(128 * c, 128 * c + 128)
            state_preds[b, sl] = res[r]["spT_o"][:, cc].T
            action_preds[b, sl] = res[r]["apT_o"][:, cc].T
            return_preds[b, sl] = res[r]["rpT_o"][:, cc].T
    return (state_preds, action_preds, return_preds)


# revision 21
# speedup vs baseline: 1.0503x; 1.0503x over previous
"""DecisionTransformer Trainium2 kernel (self-contained).

Sharding: 8 cores = 4 batch pairs x 2 sequence halves. Core r: batch r//2,
half r%2. Half 0 owns timestep blocks {0,3} (of 4 x 128-ts blocks), half 1
owns {1,2} -- balances causal-attention work exactly. Per layer each pair
AllGathers its K/V (bf16, one collective); everything else is token-local
with full weights.

Activations live transposed ([H, tokens]) so matmuls chain with zero
transposes. LayerNorm stats via PE ones-matmul column sums. Softmax skips
max-subtraction (scores are small for this model; verified vs reference).
att@V carries a ones-column per head so softmax denominators fall out of
the same matmul. bf16 matmuls, f32 PSUM accumulation, f32 residual stream.
One SPMD program for all cores; the causal-structure difference between
sequence halves is encoded in per-core mask input data.
"""
import numpy as np
import ml_dtypes

import concourse.bacc as bacc
import concourse.bass as bass
import concourse.tile as tile
from concourse import mybir

bf16 = ml_dtypes.bfloat16

B, T = 4, 512
STATE_DIM, ACT_DIM = 17, 18
H, NH, L, MAXT = 1024, 16, 4, 4096
S = 3 * T
D = H // NH
FF = 4 * H
LN_EPS = 1e-5
HP = H // 128
FFP = FF // 128

GCHUNK = [[0, 3], [1, 2]]

f32 = mybir.dt.float32
bf = mybir.dt.bfloat16
i32 = mybir.dt.int32
AF = mybir.ActivationFunctionType


def _strided(ap, start, step, count):
    return bass.AP(tensor=ap.tensor, offset=ap.offset + start,
                   ap=[list(ap.ap[0]), [step, count]])


def build_program(n_cores, own_chunks_list, uniform, n_layers=None,
                  skip_ag=False, skip_attn=False, skip_mlp=False, skip_qkv=False,
                  skip_coll=False):
    SLOTS = len(own_chunks_list)
    N_TS = 128 * SLOTS
    NT = 3 * N_TS
    NKT = 12 if uniform else 3 * (max(own_chunks_list) + 1)
    slot_valid, slot_masked, masked_pairs = [], [], []
    for j, g in enumerate(own_chunks_list):
        if uniform:
            valid = list(range(6)) if j == 0 else list(range(12))
            masked = list(range(6)) if j == 0 else list(range(6, 12))
        else:
            valid = list(range(3 * (g + 1)))
            masked = valid[-3:]
        slot_valid.append(valid)
        mm_ = {}
        for t in masked:
            mm_[t] = len(masked_pairs)
            masked_pairs.append((j, t))
        slot_masked.append(mm_)
    NMASK = len(masked_pairs)

    nc = bacc.Bacc("TRN2", target_bir_lowering=False, debug=False,
                   num_devices=n_cores)

    ts_idx = nc.dram_tensor("ts_idx", [128, SLOTS], i32, kind="ExternalInput")
    act_idx = nc.dram_tensor("act_idx", [128, SLOTS], i32, kind="ExternalInput")
    states_T = nc.dram_tensor("states_T", [STATE_DIM + 1, N_TS], f32, kind="ExternalInput")
    rtg_T = nc.dram_tensor("rtg_T", [2, N_TS], f32, kind="ExternalInput")
    masks_in = nc.dram_tensor("masks_in", [128, NMASK, 384], bf, kind="ExternalInput")
    ttab = nc.dram_tensor("ttab", [MAXT, H], f32, kind="ExternalInput")
    atab = nc.dram_tensor("atab", [ACT_DIM, H], f32, kind="ExternalInput")
    wst_in = nc.dram_tensor("wst_in", [STATE_DIM + 1, H], f32, kind="ExternalInput")
    wrtg_in = nc.dram_tensor("wrtg_in", [2, H], f32, kind="ExternalInput")
    ident_in = nc.dram_tensor("ident_in", [128, 128], f32, kind="ExternalInput")
    watt = nc.dram_tensor("watt", [L, 4, H, H], bf, kind="ExternalInput")
    w1_in = nc.dram_tensor("w1_in", [L, H, FF], bf, kind="ExternalInput")
    w2_in = nc.dram_tensor("w2_in", [L, FF, H], bf, kind="ExternalInput")
    ba_in = nc.dram_tensor("ba_in", [128, L * 4, HP], f32, kind="ExternalInput")
    b1_in = nc.dram_tensor("b1_in", [128, L, FFP], f32, kind="ExternalInput")
    b2_in = nc.dram_tensor("b2_in", [128, L, HP], f32, kind="ExternalInput")
    bv_bc = nc.dram_tensor("bv_bc", [L, 2, 512], f32, kind="ExternalInput")
    wh_in = nc.dram_tensor("wh_in", [H, 36], bf, kind="ExternalInput")
    bh_in = nc.dram_tensor("bh_in", [36], f32, kind="ExternalInput")

    spT_o = nc.dram_tensor("spT_o", [STATE_DIM, N_TS], f32, kind="ExternalOutput")
    apT_o = nc.dram_tensor("apT_o", [ACT_DIM, N_TS], f32, kind="ExternalOutput")
    rpT_o = nc.dram_tensor("rpT_o", [1, N_TS], f32, kind="ExternalOutput")

    KV_N = H * NT
    if uniform:
        cc_in_t = nc.dram_tensor("cc_in", [2 * KV_N], bf)
        cc_out_t = nc.dram_tensor("cc_out", [4 * KV_N], bf)

    NL = L if n_layers is None else n_layers
    with tile.TileContext(nc) as tc:
        with tc.tile_pool(name="sb", bufs=1) as sbp, \
             tc.tile_pool(name="rot", bufs=3) as rot, \
             tc.tile_pool(name="pp", bufs=8, space="PSUM") as pp:

            def P():
                return pp.tile([128, 512], f32, tag="mm", name="psmm")

            hT = sbp.tile([128, HP, NT], f32, tag="hT")
            h_bf = sbp.tile([128, HP, NT], bf, tag="h_bf")
            oT = sbp.tile([128, HP, NT], bf, tag="oT")
            qT = sbp.tile([128, HP, NT], bf, tag="qT")
            kT_all = sbp.tile([128, HP, 128 * NKT], bf, tag="kT_all")
            v_all = sbp.tile([128, NKT, 16 * 65], bf, tag="v_all")
            masks = sbp.tile([128, NMASK, 384], bf, tag="masks")
            ba_t = sbp.tile([128, L * 4, HP], f32, tag="ba_t")
            b1_t = sbp.tile([128, L, FFP], f32, tag="b1_t")
            b2_t = sbp.tile([128, L, HP], f32, tag="b2_t")
            bh_t = sbp.tile([36, 1], f32, tag="bh_t")
            ones_f = sbp.tile([128, 1], f32, tag="ones_f")
            ones_r = sbp.tile([1, 128], f32, tag="ones_r")
            ones_b = sbp.tile([128, 1], bf, tag="ones_b")
            eps_t = sbp.tile([1, 1], f32, tag="eps_t")
            wh_t = sbp.tile([128, HP, 36], bf, tag="wh_t")

            nc.gpsimd.dma_start(masks[:], masks_in[:])
            nc.gpsimd.dma_start(ba_t[:], ba_in[:])
            nc.gpsimd.dma_start(b1_t[:], b1_in[:])
            nc.gpsimd.dma_start(b2_t[:], b2_in[:])
            nc.gpsimd.dma_start(bh_t[:], bass.AP(tensor=bh_in, offset=0, ap=[[1, 36], [0, 1]]))
            nc.gpsimd.dma_start(wh_t[:], wh_in[:].rearrange("(kt p) o -> p kt o", p=128))
            nc.vector.memset(ones_f[:], 1.0)
            nc.vector.memset(ones_r[:], 1.0)
            nc.vector.memset(ones_b[:], 1.0)
            nc.vector.memset(eps_t[:], LN_EPS)
            for h in range(16):
                nc.vector.memset(v_all[:, :, 65 * h + 64], 1.0)

            # ---------- P0: embeddings ----------
            with tc.tile_pool(name="ep", bufs=1) as ep:
                ident = ep.tile([128, 128], f32, tag="ident")
                sb_ts = ep.tile([128, SLOTS], i32, tag="sb_ts")
                sb_ai = ep.tile([128, SLOTS], i32, tag="sb_ai")
                sb_st = ep.tile([STATE_DIM + 1, N_TS], f32, tag="sb_st")
                sb_rt = ep.tile([2, N_TS], f32, tag="sb_rt")
                wst_t = ep.tile([STATE_DIM + 1, H], f32, tag="wst_t")
                wrtg_t = ep.tile([2, H], f32, tag="wrtg_t")
                nc.gpsimd.dma_start(ident[:], ident_in[:])
                nc.gpsimd.dma_start(sb_ts[:], ts_idx[:])
                nc.gpsimd.dma_start(sb_ai[:], act_idx[:])
                nc.gpsimd.dma_start(sb_st[:], states_T[:])
                nc.gpsimd.dma_start(sb_rt[:], rtg_T[:])
                nc.gpsimd.dma_start(wst_t[:], wst_in[:])
                nc.gpsimd.dma_start(wrtg_t[:], wrtg_in[:])

                for u2 in range(SLOTS):
                    te = ep.tile([128, H], f32, tag="te", bufs=1)
                    nc.gpsimd.indirect_dma_start(
                        out=te[:], out_offset=None, in_=ttab[:],
                        in_offset=bass.IndirectOffsetOnAxis(ap=sb_ts[:, u2:u2 + 1], axis=0))
                    ae = ep.tile([128, H], f32, tag="ae", bufs=1)
                    nc.gpsimd.indirect_dma_start(
                        out=ae[:], out_offset=None, in_=atab[:],
                        in_offset=bass.IndirectOffsetOnAxis(ap=sb_ai[:, u2:u2 + 1], axis=0))
                    for ch in range(2):
                        cs = slice(512 * ch, 512 * ch + 512)
                        nat = [ep.tile([128, 512], f32, tag=f"nat{ty}",
                                       name=f"nat{ty}", bufs=2) for ty in range(3)]
                        nc.vector.tensor_add(nat[2][:], ae[:, cs], te[:, cs])
                        ps = P()
                        nc.tensor.matmul(ps[:], sb_st[:, 128 * u2:128 * u2 + 128],
                                         wst_t[:, cs], start=True, stop=True)
                        nc.vector.tensor_add(nat[1][:], ps[:], te[:, cs])
                        pr_ = P()
                        nc.tensor.matmul(pr_[:], sb_rt[:, 128 * u2:128 * u2 + 128],
                                         wrtg_t[:, cs], start=True, stop=True)
                        nc.vector.tensor_add(nat[0][:], pr_[:], te[:, cs])
                        for ty in range(3):
                            for k4 in range(4):
                                kt = 4 * ch + k4
                                pt = P()
                                nc.tensor.transpose(pt[:, 0:128],
                                                    nat[ty][:, 128 * k4:128 * k4 + 128],
                                                    ident[:])
                                dst = _strided(hT[:, kt, :], 384 * u2 + ty, 3, 128)
                                nc.vector.tensor_copy(dst, pt[:, 0:128])

            # ---------- LN (in-place on hT, refresh h_bf) ----------
            def ln_inplace():
                for c in range(SLOTS):
                    cs = slice(384 * c, 384 * c + 384)
                    ab_t = rot.tile([1, 2, 384], f32, tag="ab_t", name="ab_t", bufs=3)
                    ps1 = P()
                    ps2 = P()
                    for kt in range(HP):
                        sq = rot.tile([128, 384], bf, tag="sq", name="sq")
                        nc.scalar.activation(sq[:], hT[:, kt, cs], AF.Square)
                        nc.tensor.matmul(ps1[0:1, 0:384], ones_f[:], hT[:, kt, cs],
                                         start=(kt == 0), stop=(kt == HP - 1))
                        nc.tensor.matmul(ps2[0:1, 0:384], ones_b[:], sq[:],
                                         start=(kt == 0), stop=(kt == HP - 1))
                    mean = rot.tile([1, 384], f32, tag="mean")
                    e2 = rot.tile([1, 384], f32, tag="e2")
                    nc.scalar.activation(mean[:], ps1[0:1, 0:384], AF.Copy, scale=1.0 / H)
                    nc.scalar.activation(e2[:], ps2[0:1, 0:384], AF.Copy, scale=1.0 / H)
                    var = rot.tile([1, 384], f32, tag="var")
                    nc.vector.tensor_mul(var[:], mean[:], mean[:])
                    nc.vector.tensor_tensor(out=var[:], in0=e2[:], in1=var[:],
                                            op=mybir.AluOpType.subtract)
                    nc.scalar.activation(var[:], var[:], AF.Sqrt, bias=eps_t[:], scale=1.0)
                    nc.vector.reciprocal(ab_t[:, 0, :], var[:])
                    nc.vector.tensor_mul(ab_t[:, 1, :], mean[:], ab_t[:, 0, :])
                    nc.scalar.mul(ab_t[:, 1, :], ab_t[:, 1, :], -1.0)
                    pa = P()
                    nc.tensor.matmul(pa[:, 0:384], ones_r[:], ab_t[:, 0, :],
                                     start=True, stop=True)
                    pb = P()
                    nc.tensor.matmul(pb[:, 0:384], ones_r[:], ab_t[:, 1, :],
                                     start=True, stop=True)
                    for kt in range(HP):
                        nc.vector.tensor_mul(hT[:, kt, cs], hT[:, kt, cs], pa[:, 0:384])
                        nc.vector.tensor_add(hT[:, kt, cs], hT[:, kt, cs], pb[:, 0:384])
                        nc.vector.tensor_copy(h_bf[:, kt, cs], hT[:, kt, cs])

            ln_inplace()

            # ---------- layers ----------
            with tc.tile_pool(name="wp", bufs=4) as wp:

                def load_half(src_ap, n_mid):
                    w = wp.tile([128, n_mid, 8 * 1024 // n_mid // 2], bf,
                                tag="w", name="wtile")
                    nc.gpsimd.dma_start(w[:], src_ap)
                    return w

                def load_att_w(l, j, half):
                    # [128, HP, 512]: out-cols half*512 ... half*512+512
                    return load_half(
                        watt[l, j][:, 512 * half:512 * half + 512].rearrange(
                            "(kt p) o -> p kt o", p=128), HP)

                for l_iter in range(NL):
                    l = l_iter % L
                    # ---- QKV ----
                    for (is_q, bj) in (() if skip_qkv else ((True, 0), (False, 1))):
                        for half in range(2):
                            wt = load_att_w(l, bj, half)
                            for o4 in range(4):
                                ot = 4 * half + o4
                                for c in range(SLOTS):
                                    cs = slice(384 * c, 384 * c + 384)
                                    ps = P()
                                    for kt in range(HP):
                                        nc.tensor.matmul(
                                            ps[:, 0:384], wt[:, kt, 128 * o4:128 * o4 + 128],
                                            h_bf[:, kt, cs],
                                            start=(kt == 0), stop=(kt == HP - 1))
                                    if is_q:
                                        nc.scalar.activation(
                                            qT[:, ot, cs], ps[:, 0:384], AF.Identity,
                                            bias=ba_t[:, 4 * l + bj, ot:ot + 1])
                                    else:
                                        g = own_chunks_list[c]
                                        nc.scalar.activation(
                                            kT_all[:, ot, 384 * g:384 * g + 384],
                                            ps[:, 0:384], AF.Identity,
                                            bias=ba_t[:, 4 * l + bj, ot:ot + 1])
                    for dc in (() if skip_qkv else range(2)):
                        wv = load_att_w(l, 2, dc)
                        bvb = rot.tile([128, 512], f32, tag="tmp", name="bvb", bufs=2)
                        nc.gpsimd.dma_start(
                            bvb[:], bass.AP(tensor=bv_bc, offset=(l * 2 + dc) * 512,
                                            ap=[[0, 128], [1, 512]]))
                        for tt in range(NT // 128):
                            g = own_chunks_list[tt // 3]
                            gt = 3 * g + (tt % 3)
                            ps = P()
                            for kt in range(HP):
                                nc.tensor.matmul(ps[:], h_bf[:, kt, 128 * tt:128 * tt + 128],
                                                 wv[:, kt, :],
                                                 start=(kt == 0), stop=(kt == HP - 1))
                            dst = bass.AP(
                                tensor=v_all.tensor,
                                offset=v_all[:].offset + gt * (16 * 65) + (8 * dc) * 65,
                                ap=[list(v_all[:].ap[0]), [65, 8], [1, 64]])
                            nc.vector.tensor_tensor(
                                out=dst, in0=ps[:].rearrange("p (h d) -> p h d", h=8),
                                in1=bvb[:].rearrange("p (h d) -> p h d", h=8),
                                op=mybir.AluOpType.add)

                    # ---- AllGather K/V ----
                    if uniform and not skip_ag:
                        cc_in = cc_in_t.ap()
                        cc_out = cc_out_t.ap()
                        k_sec = cc_in[0:KV_N].rearrange("(r c) -> r c", c=NT)
                        v_sec = cc_in[KV_N:2 * KV_N].rearrange("(r c) -> r c", c=H)
                        for c in range(SLOTS):
                            g = own_chunks_list[c]
                            nc.gpsimd.dma_start(
                                k_sec[:, 384 * c:384 * c + 384].rearrange(
                                    "(kt p) q -> p kt q", p=128),
                                kT_all[:, :, 384 * g:384 * g + 384])
                            for tt in range(3):
                                src = bass.AP(
                                    tensor=v_all.tensor,
                                    offset=v_all[:].offset + (3 * g + tt) * (16 * 65),
                                    ap=[list(v_all[:].ap[0]), [65, 16], [1, 64]])
                                nc.sync.dma_start(
                                    v_sec[384 * c + 128 * tt:384 * c + 128 * tt + 128,
                                          :].rearrange("p (h d) -> p h d", h=16), src)
                        if not skip_coll:
                            groups = [[2 * i, 2 * i + 1] for i in range(n_cores // 2)]
                            nc.gpsimd.collective_compute(
                                "AllGather", mybir.AluOpType.bypass, replica_groups=groups,
                                ins=[cc_in], outs=[cc_out])
                        else:
                            nc.sync.dma_start(cc_out[0:2 * KV_N], cc_in[:])
                        for rr in range(2):
                            sec = cc_out[rr * 2 * KV_N: (rr + 1) * 2 * KV_N]
                            k_s = sec[0:KV_N].rearrange("(r c) -> r c", c=NT)
                            v_s = sec[KV_N:2 * KV_N].rearrange("(r c) -> r c", c=H)
                            for c in range(SLOTS):
                                g = GCHUNK[rr][c]
                                nc.gpsimd.dma_start(
                                    kT_all[:, :, 384 * g:384 * g + 384],
                                    k_s[:, 384 * c:384 * c + 384].rearrange(
                                        "(kt p) q -> p kt q", p=128))
                                for tt in range(3):
                                    dstv = bass.AP(
                                        tensor=v_all.tensor,
                                        offset=v_all[:].offset + (3 * g + tt) * (16 * 65),
                                        ap=[list(v_all[:].ap[0]), [65, 16], [1, 64]])
                                    nc.sync.dma_start(
                                        dstv,
                                        v_s[384 * c + 128 * tt:384 * c + 128 * tt + 128,
                                            :].rearrange("p (h d) -> p h d", h=16))

                    # ---- attention ----
                    if skip_attn:
                        nc.vector.memset(oT[:], 0.0)
                    for hp_ in (() if skip_attn else range(8)):
                        for j in range(SLOTS):
                            qs = slice(384 * j, 384 * j + 384)
                            po = [P(), P()]
                            valid = slot_valid[j]
                            for ti, t in enumerate(valid):
                                for hh in range(2):
                                    h = 2 * hp_ + hh
                                    bp = 64 * hh
                                    ps = P()
                                    nc.tensor.matmul(
                                        ps[:, 0:384],
                                        kT_all[bp:bp + 64, hp_, 128 * t:128 * t + 128],
                                        qT[bp:bp + 64, hp_, qs], start=True, stop=True)
                                    es = rot.tile([128, 384], bf, tag="es", name="es", bufs=4)
                                    nc.scalar.activation(es[:], ps[:, 0:384], AF.Exp,
                                                         scale=0.125)
                                    mi = slot_masked[j].get(t)
                                    if mi is not None:
                                        nc.vector.tensor_mul(es[:], es[:], masks[:, mi, :])
                                    nc.tensor.matmul(
                                        po[hh][0:65, 0:384],
                                        v_all[:, t, 65 * h:65 * h + 65], es[:],
                                        start=(ti == 0), stop=(ti == len(valid) - 1))
                            rcp = rot.tile([1, 2, 384], f32, tag="rcp", name="rcp", bufs=4)
                            for hh in range(2):
                                nc.vector.reciprocal(rcp[:, hh, :], po[hh][64:65, 0:384])
                            for hh in range(2):
                                pr = P()
                                nc.tensor.matmul(pr[0:64, 0:384], ones_r[:, 0:64],
                                                 rcp[:, hh, :], start=True, stop=True)
                                osl = oT[64 * hh:64 * hh + 64, hp_, qs]
                                nc.vector.tensor_copy(osl, po[hh][0:64, 0:384])
                                nc.vector.tensor_mul(osl, osl, pr[0:64, 0:384])

                    # ---- proj + residual + LN ----
                    for half in range(2):
                        wo = load_att_w(l, 3, half)
                        for o4 in range(4):
                            ot = 4 * half + o4
                            for c in range(SLOTS):
                                cs = slice(384 * c, 384 * c + 384)
                                ps = P()
                                for kt in range(HP):
                                    nc.tensor.matmul(
                                        ps[:, 0:384], wo[:, kt, 128 * o4:128 * o4 + 128],
                                        oT[:, kt, cs], start=(kt == 0), stop=(kt == HP - 1))
                                tmp = rot.tile([128, 512], f32, tag="tmp", name="tmp", bufs=2)
                                nc.scalar.activation(tmp[:, 0:384], ps[:, 0:384], AF.Identity,
                                                     bias=ba_t[:, 4 * l + 3, ot:ot + 1])
                                nc.vector.tensor_add(hT[:, ot, cs], hT[:, ot, cs],
                                                     tmp[:, 0:384])
                    ln_inplace()

                    # ---- MLP (accumulate groups directly into hT) ----
                    for grp in (() if skip_mlp else range(4)):
                        w1a = load_half(
                            w1_in[l][:, 1024 * grp:1024 * grp + 512].rearrange(
                                "(kt p) o -> p kt o", p=128), HP)
                        w1b = load_half(
                            w1_in[l][:, 1024 * grp + 512:1024 * grp + 1024].rearrange(
                                "(kt p) o -> p kt o", p=128), HP)
                        w2a = load_half(
                            w2_in[l][1024 * grp:1024 * grp + 512, :].rearrange(
                                "(ft p) o -> p ft o", p=128), 4)
                        w2b = load_half(
                            w2_in[l][1024 * grp + 512:1024 * grp + 1024, :].rearrange(
                                "(ft p) o -> p ft o", p=128), 4)
                        for c in range(NT // 256):
                            cs = slice(256 * c, 256 * c + 256)
                            mst = rot.tile([128, 8, 256], bf, tag="mst", name="mst", bufs=2)
                            for ft in range(8):
                                w1h = w1a if ft < 4 else w1b
                                f4 = ft % 4
                                pm = P()
                                for kt in range(HP):
                                    nc.tensor.matmul(
                                        pm[:, 0:256], w1h[:, kt, 128 * f4:128 * f4 + 128],
                                        h_bf[:, kt, cs],
                                        start=(kt == 0), stop=(kt == HP - 1))
                                nc.scalar.activation(
                                    mst[:, ft, :], pm[:, 0:256], AF.Gelu,
                                    bias=b1_t[:, l, 8 * grp + ft:8 * grp + ft + 1])
                            for oh in range(2):
                                ph = [P() for _ in range(4)]
                                for ft in range(8):
                                    w2h = w2a if ft < 4 else w2b
                                    f4 = ft % 4
                                    for o4 in range(4):
                                        ot = 4 * oh + o4
                                        nc.tensor.matmul(
                                            ph[o4][:, 0:256],
                                            w2h[:, f4, 128 * ot:128 * ot + 128],
                                            mst[:, ft, :], start=(ft == 0), stop=(ft == 7))
                                for o4 in range(4):
                                    ot = 4 * oh + o4
                                    nc.vector.tensor_add(hT[:, ot, cs], hT[:, ot, cs],
                                                         ph[o4][:, 0:256])
                    # b2 bias
                    for ot in range(HP):
                        nc.vector.tensor_scalar(
                            out=hT[:, ot, :], in0=hT[:, ot, :],
                            scalar1=b2_t[:, l, ot:ot + 1], scalar2=None,
                            op0=mybir.AluOpType.add)
                    ln_inplace()

                # ---------- final heads ----------
                for c in range(SLOTS):
                    ps = P()
                    for kt in range(HP):
                        rhs_a = _strided(h_bf[:, kt, :], 384 * c + 2, 3, 128)
                        nc.tensor.matmul(ps[0:36, 0:128], wh_t[:, kt, :], rhs_a,
                                         start=(kt == 0), stop=(kt == HP - 1))
                    out_s = rot.tile([36, 128], f32, tag="out_s", name="out_s")
                    nc.scalar.activation(out_s[:], ps[0:36, 0:128], AF.Identity, bias=bh_t[:])
                    nc.gpsimd.dma_start(spT_o[:, 128 * c:128 * c + 128], out_s[0:17, :])
                    nc.gpsimd.dma_start(rpT_o[:, 128 * c:128 * c + 128], out_s[35:36, :])
                    ps2 = P()
                    for kt in range(HP):
                        rhs_s = _strided(h_bf[:, kt, :], 384 * c + 1, 3, 128)
                        nc.tensor.matmul(ps2[0:36, 0:128], wh_t[:, kt, :], rhs_s,
                                         start=(kt == 0), stop=(kt == HP - 1))
                    out_s2 = rot.tile([36, 128], f32, tag="out_s", name="out_s2")
                    nc.scalar.activation(out_s2[:], ps2[0:36, 0:128], AF.Identity,
                                         bias=bh_t[:])
                    nc.gpsimd.dma_start(apT_o[:, 128 * c:128 * c + 128], out_s2[17:35, :])

    nc.compile()
    meta = dict(SLOTS=SLOTS, N_TS=N_TS, NT=NT, NKT=NKT,
                masked_pairs=masked_pairs, own_chunks_list=own_chunks_list)
    return nc, meta


# ====================== host side ======================

def make_masks(meta):
    oc = meta["own_chunks_list"]
    NM = len(meta["masked_pairs"])
    m = np.zeros((128, NM, 384), np.float32)
    for i, (j, t) in enumerate(meta["masked_pairs"]):
        g = oc[j]
        kpos = 128 * t + np.arange(128)[:, None]
        qpos = 384 * g + np.arange(384)[None, :]
        m[:, i, :] = (kpos <= qpos).astype(np.float32)
    return m.astype(bf16)


def make_core_inputs(inputs, half, b, meta, shared):
    chunks = meta["own_chunks_list"]
    ts_sl = np.concatenate([np.arange(128 * g, 128 * g + 128) for g in chunks])
    d = dict(shared)
    d["ts_idx"] = np.asarray(inputs["timesteps"])[b, ts_sl].astype(np.int32).reshape(
        len(chunks), 128).T.copy()
    d["act_idx"] = np.asarray(inputs["actions"])[b, ts_sl].astype(np.int32).reshape(
        len(chunks), 128).T.copy()
    st = np.asarray(inputs["states"], np.float32)[b, ts_sl].T  # [17, N_TS]
    d["states_T"] = np.concatenate([st, np.ones((1, st.shape[1]), np.float32)])
    rt = np.asarray(inputs["returns_to_go"], np.float32)[b, ts_sl].T  # [1, N_TS]
    d["rtg_T"] = np.concatenate([rt, np.ones((1, rt.shape[1]), np.float32)])
    return d


class SpmdRunner:
    def __init__(self, nc, n_cores):
        import jax
        from jax.sharding import Mesh, PartitionSpec
        from jax.experimental.shard_map import shard_map
        from concourse.bass2jax import (_bass_exec_p, install_neuronx_cc_hook,
                                        partition_id_tensor)
        import concourse.mybir as mybir_
        self.jax = jax
        install_neuronx_cc_hook()
        self.nc = nc
        self.n_cores = n_cores
        partition_name = nc.partition_id_tensor.name if nc.partition_id_tensor else None
        in_names, out_names, out_avals = [], [], []
        for alloc in nc.m.functions[0].allocations:
            if not isinstance(alloc, mybir_.MemoryLocationSet):
                continue
            name = alloc.memorylocations[0].name
            if alloc.kind == "ExternalInput":
                if name != partition_name:
                    in_names.append(name)
            elif alloc.kind == "ExternalOutput":
                out_names.append(name)
                out_avals.append(jax.core.ShapedArray(
                    tuple(alloc.tensor_shape), mybir_.dt.np(alloc.dtype)))
        self.in_names, self.out_names, self.out_avals = in_names, out_names, out_avals
        n_params = len(in_names)
        self._zero_outs = [np.zeros(a.shape, a.dtype) for a in out_avals]
        all_in = list(in_names) + list(out_names)
        if partition_name is not None:
            all_in.append(partition_name)

        def _body(*args):
            operands = list(args)
            if partition_name is not None:
                operands.append(partition_id_tensor())
            outs = _bass_exec_p.bind(
                *operands, out_avals=tuple(out_avals), in_names=tuple(all_in),
                out_names=tuple(out_names), lowering_input_output_aliases=(),
                sim_require_finite=False, sim_require_nnan=False, nc=nc)
            return tuple(outs)

        devices = jax.devices()[:n_cores]
        self.mesh = Mesh(np.asarray(devices), ("core",))
        in_specs = (PartitionSpec("core"),) * (n_params + len(out_names))
        out_specs = (PartitionSpec("core"),) * len(out_names)
        self._fn = jax.jit(shard_map(_body, mesh=self.mesh, in_specs=in_specs,
                                     out_specs=out_specs, check_rep=False))

    def stage_inputs(self, in_maps):
        import jax
        from jax.sharding import PartitionSpec
        n = self.n_cores
        per_core = [[np.asarray(m[name]) for name in self.in_names] for m in in_maps]
        concat = [np.concatenate([per_core[c][i] for c in range(n)], axis=0)
                  for i in range(len(self.in_names))]
        concat += [np.zeros((n * z.shape[0], *z.shape[1:]), z.dtype)
                   for z in self._zero_outs]
        sharding = jax.sharding.NamedSharding(self.mesh, PartitionSpec("core"))
        return [jax.device_put(c, sharding) for c in concat]

    def run_staged(self, staged):
        out = self._fn(*staged)
        self.jax.block_until_ready(out)
        return out

    def results(self, out_arrs):
        n = self.n_cores
        return [{name: np.asarray(out_arrs[i]).reshape(n, *self.out_avals[i].shape)[c]
                 for i, name in enumerate(self.out_names)} for c in range(n)]

    def run(self, in_maps):
        return self.results(self.run_staged(self.stage_inputs(in_maps)))


def _shared_inputs(inputs, meta):
    sh = {}
    sh["masks_in"] = make_masks(meta)
    sh["ttab"] = np.asarray(inputs["embed_time_table"], np.float32)
    sh["atab"] = np.asarray(inputs["embed_act_table"], np.float32)
    sh["wst_in"] = np.concatenate([np.asarray(inputs["Wst"], np.float32),
                                   np.asarray(inputs["bst"], np.float32)[None, :]])
    sh["wrtg_in"] = np.concatenate([np.asarray(inputs["Wrtg"], np.float32),
                                    np.asarray(inputs["brtg"], np.float32)[None, :]])
    sh["ident_in"] = np.eye(128, dtype=np.float32)
    watt = np.stack([np.asarray(inputs[k], np.float32) for k in
                     ("Wq", "Wk", "Wv", "Wo")], axis=1)
    sh["watt"] = watt.astype(bf16)
    sh["w1_in"] = np.asarray(inputs["W1"], np.float32).astype(bf16)
    sh["w2_in"] = np.asarray(inputs["W2"], np.float32).astype(bf16)
    ba = np.stack([np.asarray(inputs[k], np.float32) for k in
                   ("bq", "bk", "bv", "bo")], axis=1)          # [L,4,H]
    sh["ba_in"] = np.ascontiguousarray(
        ba.reshape(L, 4, HP, 128).transpose(3, 0, 1, 2).reshape(128, L * 4, HP))
    b1 = np.asarray(inputs["b1"], np.float32)                   # [L,FF]
    sh["b1_in"] = np.ascontiguousarray(
        b1.reshape(L, FFP, 128).transpose(2, 0, 1))
    b2 = np.asarray(inputs["b2"], np.float32)                   # [L,H]
    sh["b2_in"] = np.ascontiguousarray(
        b2.reshape(L, HP, 128).transpose(2, 0, 1))
    sh["bv_bc"] = np.ascontiguousarray(ba[:, 2, :].reshape(L, 2, 512))
    wh = np.concatenate([np.asarray(inputs["Wps"], np.float32),
                         np.asarray(inputs["Wpa"], np.float32),
                         np.asarray(inputs["Wpr"], np.float32)], axis=1)
    sh["wh_in"] = wh.astype(bf16)
    sh["bh_in"] = np.concatenate([np.asarray(inputs["bps"], np.float32),
                                  np.asarray(inputs["bpa"], np.float32),
                                  np.asarray(inputs["bpr"], np.float32)])
    return sh


_CACHE = {}


def kernel(**inputs):
    if "runner" not in _CACHE:
        nc, meta0 = build_program(8, GCHUNK[0], uniform=True)
        _CACHE["prog"] = (nc, meta0)
        _CACHE["runner"] = SpmdRunner(nc, 8)
        _CACHE["meta_by_half"] = [dict(meta0, own_chunks_list=GCHUNK[0]),
                                  dict(meta0, own_chunks_list=GCHUNK[1])]
    runner = _CACHE["runner"]
    meta_by_half = _CACHE["meta_by_half"]
    shared = [_shared_inputs(inputs, meta_by_half[0])]
    sh1 = dict(shared[0])
    sh1["masks_in"] = make_masks(meta_by_half[1])
    shared.append(sh1)
    in_maps = [make_core_inputs(inputs, r % 2, r // 2, meta_by_half[r % 2],
                                shared[r % 2]) for r in range(8)]
    staged = runner.stage_inputs(in_maps)
    _CACHE["last_staged"] = staged
    res = runner.results(runner.run_staged(staged))

    state_preds = np.zeros((B, T, STATE_DIM), np.float32)
    action_preds = np.zeros((B, T, ACT_DIM), np.float32)
    return_preds = np.zeros((B, T, 1), np.float32)
    for r in range(8):
        b, half = r // 2, r % 2
        for c, g in enumerate(GCHUNK[half]):
            sl = slice(128 * g, 128 * g + 128)
            cc = slice(128 * c, 128 * c + 128)
            state_preds[b, sl] = res[r]["spT_o"][:, cc].T
            action_preds[b, sl] = res[r]["apT_o"][:, cc].T
            return_preds[b, sl] = res[r]["rpT_o"][:, cc].T
    return (state_preds, action_preds, return_preds)


# revision 22
# speedup vs baseline: 1.0633x; 1.0123x over previous
"""DecisionTransformer Trainium2 kernel (self-contained).

Sharding: 8 cores = 4 batch pairs x 2 sequence halves. Core r: batch r//2,
half r%2. Half 0 owns timestep blocks {0,3} (of 4 x 128-ts blocks), half 1
owns {1,2} -- balances causal-attention work exactly. Per layer each pair
AllGathers its K/V (bf16, one collective); everything else is token-local
with full weights.

Activations live transposed ([H, tokens]) so matmuls chain with zero
transposes. LayerNorm stats via PE ones-matmul column sums. Softmax skips
max-subtraction (scores are small for this model; verified vs reference).
att@V carries a ones-column per head so softmax denominators fall out of
the same matmul. bf16 matmuls, f32 PSUM accumulation, f32 residual stream.
One SPMD program for all cores; the causal-structure difference between
sequence halves is encoded in per-core mask input data.
"""
import numpy as np
import ml_dtypes

import concourse.bacc as bacc
import concourse.bass as bass
import concourse.tile as tile
from concourse import mybir

bf16 = ml_dtypes.bfloat16

B, T = 4, 512
STATE_DIM, ACT_DIM = 17, 18
H, NH, L, MAXT = 1024, 16, 4, 4096
S = 3 * T
D = H // NH
FF = 4 * H
LN_EPS = 1e-5
HP = H // 128
FFP = FF // 128

GCHUNK = [[0, 3], [1, 2]]

f32 = mybir.dt.float32
bf = mybir.dt.bfloat16
i32 = mybir.dt.int32
AF = mybir.ActivationFunctionType


def _strided(ap, start, step, count):
    return bass.AP(tensor=ap.tensor, offset=ap.offset + start,
                   ap=[list(ap.ap[0]), [step, count]])


def build_program(n_cores, own_chunks_list, uniform, n_layers=None,
                  skip_ag=False, skip_attn=False, skip_mlp=False, skip_qkv=False,
                  skip_coll=False):
    SLOTS = len(own_chunks_list)
    N_TS = 128 * SLOTS
    NT = 3 * N_TS
    NKT = 12 if uniform else 3 * (max(own_chunks_list) + 1)
    slot_valid, slot_masked, masked_pairs = [], [], []
    for j, g in enumerate(own_chunks_list):
        if uniform:
            valid = list(range(6)) if j == 0 else list(range(12))
            masked = list(range(6)) if j == 0 else list(range(6, 12))
        else:
            valid = list(range(3 * (g + 1)))
            masked = valid[-3:]
        slot_valid.append(valid)
        mm_ = {}
        for t in masked:
            mm_[t] = len(masked_pairs)
            masked_pairs.append((j, t))
        slot_masked.append(mm_)
    NMASK = len(masked_pairs)

    nc = bacc.Bacc("TRN2", target_bir_lowering=False, debug=False,
                   num_devices=n_cores)

    ts_idx = nc.dram_tensor("ts_idx", [128, SLOTS], i32, kind="ExternalInput")
    act_idx = nc.dram_tensor("act_idx", [128, SLOTS], i32, kind="ExternalInput")
    states_T = nc.dram_tensor("states_T", [STATE_DIM + 1, N_TS], f32, kind="ExternalInput")
    rtg_T = nc.dram_tensor("rtg_T", [2, N_TS], f32, kind="ExternalInput")
    masks_in = nc.dram_tensor("masks_in", [128, NMASK, 384], bf, kind="ExternalInput")
    ttab = nc.dram_tensor("ttab", [MAXT, H], f32, kind="ExternalInput")
    atab = nc.dram_tensor("atab", [ACT_DIM, H], f32, kind="ExternalInput")
    wst_in = nc.dram_tensor("wst_in", [STATE_DIM + 1, H], f32, kind="ExternalInput")
    wrtg_in = nc.dram_tensor("wrtg_in", [2, H], f32, kind="ExternalInput")
    ident_in = nc.dram_tensor("ident_in", [128, 128], f32, kind="ExternalInput")
    watt = nc.dram_tensor("watt", [L, 4, H, H], bf, kind="ExternalInput")
    w1_in = nc.dram_tensor("w1_in", [L, H, FF], bf, kind="ExternalInput")
    w2_in = nc.dram_tensor("w2_in", [L, FF, H], bf, kind="ExternalInput")
    ba_in = nc.dram_tensor("ba_in", [128, L * 4, HP], f32, kind="ExternalInput")
    b1_in = nc.dram_tensor("b1_in", [128, L, FFP], f32, kind="ExternalInput")
    b2_in = nc.dram_tensor("b2_in", [128, L, HP], f32, kind="ExternalInput")
    bv_bc = nc.dram_tensor("bv_bc", [L, 2, 512], f32, kind="ExternalInput")
    wh_in = nc.dram_tensor("wh_in", [H, 36], bf, kind="ExternalInput")
    bh_in = nc.dram_tensor("bh_in", [36], f32, kind="ExternalInput")

    spT_o = nc.dram_tensor("spT_o", [STATE_DIM, N_TS], f32, kind="ExternalOutput")
    apT_o = nc.dram_tensor("apT_o", [ACT_DIM, N_TS], f32, kind="ExternalOutput")
    rpT_o = nc.dram_tensor("rpT_o", [1, N_TS], f32, kind="ExternalOutput")

    KV_N = H * NT
    if uniform:
        cc_in_t = nc.dram_tensor("cc_in", [2 * KV_N], bf)
        cc_out_t = nc.dram_tensor("cc_out", [4 * KV_N], bf)

    NL = L if n_layers is None else n_layers
    with tile.TileContext(nc) as tc:
        with tc.tile_pool(name="sb", bufs=1) as sbp, \
             tc.tile_pool(name="rot", bufs=3) as rot, \
             tc.tile_pool(name="pp", bufs=8, space="PSUM") as pp:

            def P():
                return pp.tile([128, 512], f32, tag="mm", name="psmm")

            hT = sbp.tile([128, HP, NT], f32, tag="hT")
            h_bf = sbp.tile([128, HP, NT], bf, tag="h_bf")
            oT = sbp.tile([128, HP, NT], bf, tag="oT")
            qT = sbp.tile([128, HP, NT], bf, tag="qT")
            kT_all = sbp.tile([128, HP, 128 * NKT], bf, tag="kT_all")
            v_all = sbp.tile([128, NKT, 16 * 65], bf, tag="v_all")
            masks = sbp.tile([128, NMASK, 384], bf, tag="masks")
            ba_t = sbp.tile([128, L * 4, HP], f32, tag="ba_t")
            b1_t = sbp.tile([128, L, FFP], f32, tag="b1_t")
            b2_t = sbp.tile([128, L, HP], f32, tag="b2_t")
            bh_t = sbp.tile([36, 1], f32, tag="bh_t")
            ones_f = sbp.tile([128, 1], f32, tag="ones_f")
            ones_r = sbp.tile([1, 128], f32, tag="ones_r")
            ones_b = sbp.tile([128, 1], bf, tag="ones_b")
            eps_t = sbp.tile([1, 1], f32, tag="eps_t")
            wh_t = sbp.tile([128, HP, 36], bf, tag="wh_t")

            nc.gpsimd.dma_start(masks[:], masks_in[:])
            nc.gpsimd.dma_start(ba_t[:], ba_in[:])
            nc.gpsimd.dma_start(b1_t[:], b1_in[:])
            nc.gpsimd.dma_start(b2_t[:], b2_in[:])
            nc.gpsimd.dma_start(bh_t[:], bass.AP(tensor=bh_in, offset=0, ap=[[1, 36], [0, 1]]))
            nc.gpsimd.dma_start(wh_t[:], wh_in[:].rearrange("(kt p) o -> p kt o", p=128))
            nc.vector.memset(ones_f[:], 1.0)
            nc.vector.memset(ones_r[:], 1.0)
            nc.vector.memset(ones_b[:], 1.0)
            nc.vector.memset(eps_t[:], LN_EPS)
            for h in range(16):
                nc.vector.memset(v_all[:, :, 65 * h + 64], 1.0)

            # ---------- P0: embeddings ----------
            with tc.tile_pool(name="ep", bufs=1) as ep:
                ident = ep.tile([128, 128], f32, tag="ident")
                sb_ts = ep.tile([128, SLOTS], i32, tag="sb_ts")
                sb_ai = ep.tile([128, SLOTS], i32, tag="sb_ai")
                sb_st = ep.tile([STATE_DIM + 1, N_TS], f32, tag="sb_st")
                sb_rt = ep.tile([2, N_TS], f32, tag="sb_rt")
                wst_t = ep.tile([STATE_DIM + 1, H], f32, tag="wst_t")
                wrtg_t = ep.tile([2, H], f32, tag="wrtg_t")
                nc.gpsimd.dma_start(ident[:], ident_in[:])
                nc.gpsimd.dma_start(sb_ts[:], ts_idx[:])
                nc.gpsimd.dma_start(sb_ai[:], act_idx[:])
                nc.gpsimd.dma_start(sb_st[:], states_T[:])
                nc.gpsimd.dma_start(sb_rt[:], rtg_T[:])
                nc.gpsimd.dma_start(wst_t[:], wst_in[:])
                nc.gpsimd.dma_start(wrtg_t[:], wrtg_in[:])

                for u2 in range(SLOTS):
                    te = ep.tile([128, H], f32, tag="te", bufs=1)
                    nc.gpsimd.indirect_dma_start(
                        out=te[:], out_offset=None, in_=ttab[:],
                        in_offset=bass.IndirectOffsetOnAxis(ap=sb_ts[:, u2:u2 + 1], axis=0))
                    ae = ep.tile([128, H], f32, tag="ae", bufs=1)
                    nc.gpsimd.indirect_dma_start(
                        out=ae[:], out_offset=None, in_=atab[:],
                        in_offset=bass.IndirectOffsetOnAxis(ap=sb_ai[:, u2:u2 + 1], axis=0))
                    for ch in range(2):
                        cs = slice(512 * ch, 512 * ch + 512)
                        nat = [ep.tile([128, 512], f32, tag=f"nat{ty}",
                                       name=f"nat{ty}", bufs=2) for ty in range(3)]
                        nc.vector.tensor_add(nat[2][:], ae[:, cs], te[:, cs])
                        ps = P()
                        nc.tensor.matmul(ps[:], sb_st[:, 128 * u2:128 * u2 + 128],
                                         wst_t[:, cs], start=True, stop=True)
                        nc.vector.tensor_add(nat[1][:], ps[:], te[:, cs])
                        pr_ = P()
                        nc.tensor.matmul(pr_[:], sb_rt[:, 128 * u2:128 * u2 + 128],
                                         wrtg_t[:, cs], start=True, stop=True)
                        nc.vector.tensor_add(nat[0][:], pr_[:], te[:, cs])
                        for ty in range(3):
                            for k4 in range(4):
                                kt = 4 * ch + k4
                                pt = P()
                                nc.tensor.transpose(pt[:, 0:128],
                                                    nat[ty][:, 128 * k4:128 * k4 + 128],
                                                    ident[:])
                                dst = _strided(hT[:, kt, :], 384 * u2 + ty, 3, 128)
                                nc.vector.tensor_copy(dst, pt[:, 0:128])

            # ---------- LN (in-place on hT, refresh h_bf) ----------
            def ln_inplace():
                for c in range(SLOTS):
                    cs = slice(384 * c, 384 * c + 384)
                    ab_t = rot.tile([1, 2, 384], f32, tag="ab_t", name="ab_t", bufs=3)
                    ps1 = P()
                    ps2 = P()
                    for kt in range(HP):
                        sq = rot.tile([128, 384], bf, tag="sq", name="sq")
                        nc.scalar.activation(sq[:], hT[:, kt, cs], AF.Square)
                        nc.tensor.matmul(ps1[0:1, 0:384], ones_f[:], hT[:, kt, cs],
                                         start=(kt == 0), stop=(kt == HP - 1))
                        nc.tensor.matmul(ps2[0:1, 0:384], ones_b[:], sq[:],
                                         start=(kt == 0), stop=(kt == HP - 1))
                    mean = rot.tile([1, 384], f32, tag="mean")
                    e2 = rot.tile([1, 384], f32, tag="e2")
                    nc.scalar.activation(mean[:], ps1[0:1, 0:384], AF.Copy, scale=1.0 / H)
                    nc.scalar.activation(e2[:], ps2[0:1, 0:384], AF.Copy, scale=1.0 / H)
                    var = rot.tile([1, 384], f32, tag="var")
                    nc.vector.tensor_mul(var[:], mean[:], mean[:])
                    nc.vector.tensor_tensor(out=var[:], in0=e2[:], in1=var[:],
                                            op=mybir.AluOpType.subtract)
                    nc.scalar.activation(var[:], var[:], AF.Sqrt, bias=eps_t[:], scale=1.0)
                    nc.vector.reciprocal(ab_t[:, 0, :], var[:])
                    nc.vector.tensor_mul(ab_t[:, 1, :], mean[:], ab_t[:, 0, :])
                    nc.scalar.mul(ab_t[:, 1, :], ab_t[:, 1, :], -1.0)
                    pa = P()
                    nc.tensor.matmul(pa[:, 0:384], ones_r[:], ab_t[:, 0, :],
                                     start=True, stop=True)
                    pb = P()
                    nc.tensor.matmul(pb[:, 0:384], ones_r[:], ab_t[:, 1, :],
                                     start=True, stop=True)
                    for kt in range(HP):
                        nc.vector.tensor_mul(hT[:, kt, cs], hT[:, kt, cs], pa[:, 0:384])
                        nc.vector.tensor_add(hT[:, kt, cs], hT[:, kt, cs], pb[:, 0:384])
                        nc.vector.tensor_copy(h_bf[:, kt, cs], hT[:, kt, cs])

            ln_inplace()

            # ---------- layers ----------
            with tc.tile_pool(name="wp", bufs=4) as wp:

                def load_half(src_ap, n_mid):
                    w = wp.tile([128, n_mid, 8 * 1024 // n_mid // 2], bf,
                                tag="w", name="wtile")
                    nc.gpsimd.dma_start(w[:], src_ap)
                    return w

                def load_att_w(l, j, half):
                    # [128, HP, 512]: out-cols half*512 ... half*512+512
                    return load_half(
                        watt[l, j][:, 512 * half:512 * half + 512].rearrange(
                            "(kt p) o -> p kt o", p=128), HP)

                for l_iter in range(NL):
                    l = l_iter % L
                    # ---- QKV ----
                    for (is_q, bj) in (() if skip_qkv else ((True, 0), (False, 1))):
                        for half in range(2):
                            wt = load_att_w(l, bj, half)
                            for o4 in range(4):
                                ot = 4 * half + o4
                                for c in range(SLOTS):
                                    cs = slice(384 * c, 384 * c + 384)
                                    ps = P()
                                    for kt in range(HP):
                                        nc.tensor.matmul(
                                            ps[:, 0:384], wt[:, kt, 128 * o4:128 * o4 + 128],
                                            h_bf[:, kt, cs],
                                            start=(kt == 0), stop=(kt == HP - 1))
                                    if is_q:
                                        nc.scalar.activation(
                                            qT[:, ot, cs], ps[:, 0:384], AF.Identity,
                                            bias=ba_t[:, 4 * l + bj, ot:ot + 1])
                                    else:
                                        g = own_chunks_list[c]
                                        nc.scalar.activation(
                                            kT_all[:, ot, 384 * g:384 * g + 384],
                                            ps[:, 0:384], AF.Identity,
                                            bias=ba_t[:, 4 * l + bj, ot:ot + 1])
                    for dc in (() if skip_qkv else range(2)):
                        wv = load_att_w(l, 2, dc)
                        bvb = rot.tile([128, 512], f32, tag="tmp", name="bvb", bufs=2)
                        nc.gpsimd.dma_start(
                            bvb[:], bass.AP(tensor=bv_bc, offset=(l * 2 + dc) * 512,
                                            ap=[[0, 128], [1, 512]]))
                        for tt in range(NT // 128):
                            g = own_chunks_list[tt // 3]
                            gt = 3 * g + (tt % 3)
                            ps = P()
                            for kt in range(HP):
                                nc.tensor.matmul(ps[:], h_bf[:, kt, 128 * tt:128 * tt + 128],
                                                 wv[:, kt, :],
                                                 start=(kt == 0), stop=(kt == HP - 1))
                            dst = bass.AP(
                                tensor=v_all.tensor,
                                offset=v_all[:].offset + gt * (16 * 65) + (8 * dc) * 65,
                                ap=[list(v_all[:].ap[0]), [65, 8], [1, 64]])
                            nc.vector.tensor_tensor(
                                out=dst, in0=ps[:].rearrange("p (h d) -> p h d", h=8),
                                in1=bvb[:].rearrange("p (h d) -> p h d", h=8),
                                op=mybir.AluOpType.add)

                    # ---- AllGather K/V ----
                    if uniform and not skip_ag:
                        cc_in = cc_in_t.ap()
                        cc_out = cc_out_t.ap()
                        k_sec = cc_in[0:KV_N].rearrange("(r c) -> r c", c=NT)
                        v_sec = cc_in[KV_N:2 * KV_N].rearrange("(r c) -> r c", c=H)
                        for c in range(SLOTS):
                            g = own_chunks_list[c]
                            nc.gpsimd.dma_start(
                                k_sec[:, 384 * c:384 * c + 384].rearrange(
                                    "(kt p) q -> p kt q", p=128),
                                kT_all[:, :, 384 * g:384 * g + 384])
                            for tt in range(3):
                                src = bass.AP(
                                    tensor=v_all.tensor,
                                    offset=v_all[:].offset + (3 * g + tt) * (16 * 65),
                                    ap=[list(v_all[:].ap[0]), [65, 16], [1, 64]])
                                nc.sync.dma_start(
                                    v_sec[384 * c + 128 * tt:384 * c + 128 * tt + 128,
                                          :].rearrange("p (h d) -> p h d", h=16), src)
                        if not skip_coll:
                            groups = [[2 * i, 2 * i + 1] for i in range(n_cores // 2)]
                            nc.gpsimd.collective_compute(
                                "AllGather", mybir.AluOpType.bypass, replica_groups=groups,
                                ins=[cc_in], outs=[cc_out])
                        else:
                            nc.sync.dma_start(cc_out[0:2 * KV_N], cc_in[:])
                        for rr in range(2):
                            sec = cc_out[rr * 2 * KV_N: (rr + 1) * 2 * KV_N]
                            k_s = sec[0:KV_N].rearrange("(r c) -> r c", c=NT)
                            v_s = sec[KV_N:2 * KV_N].rearrange("(r c) -> r c", c=H)
                            for c in range(SLOTS):
                                g = GCHUNK[rr][c]
                                nc.gpsimd.dma_start(
                                    kT_all[:, :, 384 * g:384 * g + 384],
                                    k_s[:, 384 * c:384 * c + 384].rearrange(
                                        "(kt p) q -> p kt q", p=128))
                                for tt in range(3):
                                    dstv = bass.AP(
                                        tensor=v_all.tensor,
                                        offset=v_all[:].offset + (3 * g + tt) * (16 * 65),
                                        ap=[list(v_all[:].ap[0]), [65, 16], [1, 64]])
                                    nc.sync.dma_start(
                                        dstv,
                                        v_s[384 * c + 128 * tt:384 * c + 128 * tt + 128,
                                            :].rearrange("p (h d) -> p h d", h=16))

                    # ---- attention ----
                    if skip_attn:
                        nc.vector.memset(oT[:], 0.0)
                    for hp_ in (() if skip_attn else range(8)):
                        pos = {j: [P(), P()] for j in range(SLOTS)}
                        nmax = max(len(slot_valid[j]) for j in range(SLOTS))
                        for ti in range(nmax):
                            for j in range(SLOTS):
                                valid = slot_valid[j]
                                if ti >= len(valid):
                                    continue
                                t = valid[ti]
                                qs = slice(384 * j, 384 * j + 384)
                                for hh in range(2):
                                    h = 2 * hp_ + hh
                                    bp = 64 * hh
                                    ps = P()
                                    nc.tensor.matmul(
                                        ps[:, 0:384],
                                        kT_all[bp:bp + 64, hp_, 128 * t:128 * t + 128],
                                        qT[bp:bp + 64, hp_, qs], start=True, stop=True)
                                    es = rot.tile([128, 384], bf, tag="es", name="es", bufs=4)
                                    nc.scalar.activation(es[:], ps[:, 0:384], AF.Exp,
                                                         scale=0.125)
                                    mi = slot_masked[j].get(t)
                                    if mi is not None:
                                        nc.vector.tensor_mul(es[:], es[:], masks[:, mi, :])
                                    nc.tensor.matmul(
                                        pos[j][hh][0:65, 0:384],
                                        v_all[:, t, 65 * h:65 * h + 65], es[:],
                                        start=(ti == 0), stop=(ti == len(valid) - 1))
                        for j in (() if skip_attn else range(SLOTS)):
                            qs = slice(384 * j, 384 * j + 384)
                            po = pos[j]
                            rcp = rot.tile([1, 2, 384], f32, tag="rcp", name="rcp", bufs=4)
                            for hh in range(2):
                                nc.vector.reciprocal(rcp[:, hh, :], po[hh][64:65, 0:384])
                            for hh in range(2):
                                pr = P()
                                nc.tensor.matmul(pr[0:64, 0:384], ones_r[:, 0:64],
                                                 rcp[:, hh, :], start=True, stop=True)
                                osl = oT[64 * hh:64 * hh + 64, hp_, qs]
                                nc.vector.tensor_copy(osl, po[hh][0:64, 0:384])
                                nc.vector.tensor_mul(osl, osl, pr[0:64, 0:384])

                    # ---- proj + residual + LN ----
                    for half in range(2):
                        wo = load_att_w(l, 3, half)
                        for o4 in range(4):
                            ot = 4 * half + o4
                            for c in range(SLOTS):
                                cs = slice(384 * c, 384 * c + 384)
                                ps = P()
                                for kt in range(HP):
                                    nc.tensor.matmul(
                                        ps[:, 0:384], wo[:, kt, 128 * o4:128 * o4 + 128],
                                        oT[:, kt, cs], start=(kt == 0), stop=(kt == HP - 1))
                                tmp = rot.tile([128, 512], f32, tag="tmp", name="tmp", bufs=2)
                                nc.scalar.activation(tmp[:, 0:384], ps[:, 0:384], AF.Identity,
                                                     bias=ba_t[:, 4 * l + 3, ot:ot + 1])
                                nc.vector.tensor_add(hT[:, ot, cs], hT[:, ot, cs],
                                                     tmp[:, 0:384])
                    ln_inplace()

                    # ---- MLP (accumulate groups directly into hT) ----
                    for grp in (() if skip_mlp else range(4)):
                        w1a = load_half(
                            w1_in[l][:, 1024 * grp:1024 * grp + 512].rearrange(
                                "(kt p) o -> p kt o", p=128), HP)
                        w1b = load_half(
                            w1_in[l][:, 1024 * grp + 512:1024 * grp + 1024].rearrange(
                                "(kt p) o -> p kt o", p=128), HP)
                        w2a = load_half(
                            w2_in[l][1024 * grp:1024 * grp + 512, :].rearrange(
                                "(ft p) o -> p ft o", p=128), 4)
                        w2b = load_half(
                            w2_in[l][1024 * grp + 512:1024 * grp + 1024, :].rearrange(
                                "(ft p) o -> p ft o", p=128), 4)
                        for c in range(NT // 256):
                            cs = slice(256 * c, 256 * c + 256)
                            mst = rot.tile([128, 8, 256], bf, tag="mst", name="mst", bufs=2)
                            for ft in range(8):
                                w1h = w1a if ft < 4 else w1b
                                f4 = ft % 4
                                pm = P()
                                for kt in range(HP):
                                    nc.tensor.matmul(
                                        pm[:, 0:256], w1h[:, kt, 128 * f4:128 * f4 + 128],
                                        h_bf[:, kt, cs],
                                        start=(kt == 0), stop=(kt == HP - 1))
                                nc.scalar.activation(
                                    mst[:, ft, :], pm[:, 0:256], AF.Gelu,
                                    bias=b1_t[:, l, 8 * grp + ft:8 * grp + ft + 1])
                            for oh in range(2):
                                ph = [P() for _ in range(4)]
                                for ft in range(8):
                                    w2h = w2a if ft < 4 else w2b
                                    f4 = ft % 4
                                    for o4 in range(4):
                                        ot = 4 * oh + o4
                                        nc.tensor.matmul(
                                            ph[o4][:, 0:256],
                                            w2h[:, f4, 128 * ot:128 * ot + 128],
                                            mst[:, ft, :], start=(ft == 0), stop=(ft == 7))
                                for o4 in range(4):
                                    ot = 4 * oh + o4
                                    nc.vector.tensor_add(hT[:, ot, cs], hT[:, ot, cs],
                                                         ph[o4][:, 0:256])
                    # b2 bias
                    for ot in range(HP):
                        nc.vector.tensor_scalar(
                            out=hT[:, ot, :], in0=hT[:, ot, :],
                            scalar1=b2_t[:, l, ot:ot + 1], scalar2=None,
                            op0=mybir.AluOpType.add)
                    ln_inplace()

                # ---------- final heads ----------
                for c in range(SLOTS):
                    ps = P()
                    for kt in range(HP):
                        rhs_a = _strided(h_bf[:, kt, :], 384 * c + 2, 3, 128)
                        nc.tensor.matmul(ps[0:36, 0:128], wh_t[:, kt, :], rhs_a,
                                         start=(kt == 0), stop=(kt == HP - 1))
                    out_s = rot.tile([36, 128], f32, tag="out_s", name="out_s")
                    nc.scalar.activation(out_s[:], ps[0:36, 0:128], AF.Identity, bias=bh_t[:])
                    nc.gpsimd.dma_start(spT_o[:, 128 * c:128 * c + 128], out_s[0:17, :])
                    nc.gpsimd.dma_start(rpT_o[:, 128 * c:128 * c + 128], out_s[35:36, :])
                    ps2 = P()
                    for kt in range(HP):
                        rhs_s = _strided(h_bf[:, kt, :], 384 * c + 1, 3, 128)
                        nc.tensor.matmul(ps2[0:36, 0:128], wh_t[:, kt, :], rhs_s,
                                         start=(kt == 0), stop=(kt == HP - 1))
                    out_s2 = rot.tile([36, 128], f32, tag="out_s", name="out_s2")
                    nc.scalar.activation(out_s2[:], ps2[0:36, 0:128], AF.Identity,
                                         bias=bh_t[:])
                    nc.gpsimd.dma_start(apT_o[:, 128 * c:128 * c + 128], out_s2[17:35, :])

    nc.compile()
    meta = dict(SLOTS=SLOTS, N_TS=N_TS, NT=NT, NKT=NKT,
                masked_pairs=masked_pairs, own_chunks_list=own_chunks_list)
    return nc, meta


# ====================== host side ======================

def make_masks(meta):
    oc = meta["own_chunks_list"]
    NM = len(meta["masked_pairs"])
    m = np.zeros((128, NM, 384), np.float32)
    for i, (j, t) in enumerate(meta["masked_pairs"]):
        g = oc[j]
        kpos = 128 * t + np.arange(128)[:, None]
        qpos = 384 * g + np.arange(384)[None, :]
        m[:, i, :] = (kpos <= qpos).astype(np.float32)
    return m.astype(bf16)


def make_core_inputs(inputs, half, b, meta, shared):
    chunks = meta["own_chunks_list"]
    ts_sl = np.concatenate([np.arange(128 * g, 128 * g + 128) for g in chunks])
    d = dict(shared)
    d["ts_idx"] = np.asarray(inputs["timesteps"])[b, ts_sl].astype(np.int32).reshape(
        len(chunks), 128).T.copy()
    d["act_idx"] = np.asarray(inputs["actions"])[b, ts_sl].astype(np.int32).reshape(
        len(chunks), 128).T.copy()
    st = np.asarray(inputs["states"], np.float32)[b, ts_sl].T  # [17, N_TS]
    d["states_T"] = np.concatenate([st, np.ones((1, st.shape[1]), np.float32)])
    rt = np.asarray(inputs["returns_to_go"], np.float32)[b, ts_sl].T  # [1, N_TS]
    d["rtg_T"] = np.concatenate([rt, np.ones((1, rt.shape[1]), np.float32)])
    return d


class SpmdRunner:
    def __init__(self, nc, n_cores):
        import jax
        from jax.sharding import Mesh, PartitionSpec
        from jax.experimental.shard_map import shard_map
        from concourse.bass2jax import (_bass_exec_p, install_neuronx_cc_hook,
                                        partition_id_tensor)
        import concourse.mybir as mybir_
        self.jax = jax
        install_neuronx_cc_hook()
        self.nc = nc
        self.n_cores = n_cores
        partition_name = nc.partition_id_tensor.name if nc.partition_id_tensor else None
        in_names, out_names, out_avals = [], [], []
        for alloc in nc.m.functions[0].allocations:
            if not isinstance(alloc, mybir_.MemoryLocationSet):
                continue
            name = alloc.memorylocations[0].name
            if alloc.kind == "ExternalInput":
                if name != partition_name:
                    in_names.append(name)
            elif alloc.kind == "ExternalOutput":
                out_names.append(name)
                out_avals.append(jax.core.ShapedArray(
                    tuple(alloc.tensor_shape), mybir_.dt.np(alloc.dtype)))
        self.in_names, self.out_names, self.out_avals = in_names, out_names, out_avals
        n_params = len(in_names)
        self._zero_outs = [np.zeros(a.shape, a.dtype) for a in out_avals]
        all_in = list(in_names) + list(out_names)
        if partition_name is not None:
            all_in.append(partition_name)

        def _body(*args):
            operands = list(args)
            if partition_name is not None:
                operands.append(partition_id_tensor())
            outs = _bass_exec_p.bind(
                *operands, out_avals=tuple(out_avals), in_names=tuple(all_in),
                out_names=tuple(out_names), lowering_input_output_aliases=(),
                sim_require_finite=False, sim_require_nnan=False, nc=nc)
            return tuple(outs)

        devices = jax.devices()[:n_cores]
        self.mesh = Mesh(np.asarray(devices), ("core",))
        in_specs = (PartitionSpec("core"),) * (n_params + len(out_names))
        out_specs = (PartitionSpec("core"),) * len(out_names)
        self._fn = jax.jit(shard_map(_body, mesh=self.mesh, in_specs=in_specs,
                                     out_specs=out_specs, check_rep=False))

    def stage_inputs(self, in_maps):
        import jax
        from jax.sharding import PartitionSpec
        n = self.n_cores
        per_core = [[np.asarray(m[name]) for name in self.in_names] for m in in_maps]
        concat = [np.concatenate([per_core[c][i] for c in range(n)], axis=0)
                  for i in range(len(self.in_names))]
        concat += [np.zeros((n * z.shape[0], *z.shape[1:]), z.dtype)
                   for z in self._zero_outs]
        sharding = jax.sharding.NamedSharding(self.mesh, PartitionSpec("core"))
        return [jax.device_put(c, sharding) for c in concat]

    def run_staged(self, staged):
        out = self._fn(*staged)
        self.jax.block_until_ready(out)
        return out

    def results(self, out_arrs):
        n = self.n_cores
        return [{name: np.asarray(out_arrs[i]).reshape(n, *self.out_avals[i].shape)[c]
                 for i, name in enumerate(self.out_names)} for c in range(n)]

    def run(self, in_maps):
        return self.results(self.run_staged(self.stage_inputs(in_maps)))


def _shared_inputs(inputs, meta):
    sh = {}
    sh["masks_in"] = make_masks(meta)
    sh["ttab"] = np.asarray(inputs["embed_time_table"], np.float32)
    sh["atab"] = np.asarray(inputs["embed_act_table"], np.float32)
    sh["wst_in"] = np.concatenate([np.asarray(inputs["Wst"], np.float32),
                                   np.asarray(inputs["bst"], np.float32)[None, :]])
    sh["wrtg_in"] = np.concatenate([np.asarray(inputs["Wrtg"], np.float32),
                                    np.asarray(inputs["brtg"], np.float32)[None, :]])
    sh["ident_in"] = np.eye(128, dtype=np.float32)
    watt = np.stack([np.asarray(inputs[k], np.float32) for k in
                     ("Wq", "Wk", "Wv", "Wo")], axis=1)
    sh["watt"] = watt.astype(bf16)
    sh["w1_in"] = np.asarray(inputs["W1"], np.float32).astype(bf16)
    sh["w2_in"] = np.asarray(inputs["W2"], np.float32).astype(bf16)
    ba = np.stack([np.asarray(inputs[k], np.float32) for k in
                   ("bq", "bk", "bv", "bo")], axis=1)          # [L,4,H]
    sh["ba_in"] = np.ascontiguousarray(
        ba.reshape(L, 4, HP, 128).transpose(3, 0, 1, 2).reshape(128, L * 4, HP))
    b1 = np.asarray(inputs["b1"], np.float32)                   # [L,FF]
    sh["b1_in"] = np.ascontiguousarray(
        b1.reshape(L, FFP, 128).transpose(2, 0, 1))
    b2 = np.asarray(inputs["b2"], np.float32)                   # [L,H]
    sh["b2_in"] = np.ascontiguousarray(
        b2.reshape(L, HP, 128).transpose(2, 0, 1))
    sh["bv_bc"] = np.ascontiguousarray(ba[:, 2, :].reshape(L, 2, 512))
    wh = np.concatenate([np.asarray(inputs["Wps"], np.float32),
                         np.asarray(inputs["Wpa"], np.float32),
                         np.asarray(inputs["Wpr"], np.float32)], axis=1)
    sh["wh_in"] = wh.astype(bf16)
    sh["bh_in"] = np.concatenate([np.asarray(inputs["bps"], np.float32),
                                  np.asarray(inputs["bpa"], np.float32),
                                  np.asarray(inputs["bpr"], np.float32)])
    return sh


_CACHE = {}


def kernel(**inputs):
    if "runner" not in _CACHE:
        nc, meta0 = build_program(8, GCHUNK[0], uniform=True)
        _CACHE["prog"] = (nc, meta0)
        _CACHE["runner"] = SpmdRunner(nc, 8)
        _CACHE["meta_by_half"] = [dict(meta0, own_chunks_list=GCHUNK[0]),
                                  dict(meta0, own_chunks_list=GCHUNK[1])]
    runner = _CACHE["runner"]
    meta_by_half = _CACHE["meta_by_half"]
    shared = [_shared_inputs(inputs, meta_by_half[0])]
    sh1 = dict(shared[0])
    sh1["masks_in"] = make_masks(meta_by_half[1])
    shared.append(sh1)
    in_maps = [make_core_inputs(inputs, r % 2, r // 2, meta_by_half[r % 2],
                                shared[r % 2]) for r in range(8)]
    staged = runner.stage_inputs(in_maps)
    _CACHE["last_staged"] = staged
    res = runner.results(runner.run_staged(staged))

    state_preds = np.zeros((B, T, STATE_DIM), np.float32)
    action_preds = np.zeros((B, T, ACT_DIM), np.float32)
    return_preds = np.zeros((B, T, 1), np.float32)
    for r in range(8):
        b, half = r // 2, r % 2
        for c, g in enumerate(GCHUNK[half]):
            sl = slice(128 * g, 128 * g + 128)
            cc = slice(128 * c, 128 * c + 128)
            state_preds[b, sl] = res[r]["spT_o"][:, cc].T
            action_preds[b, sl] = res[r]["apT_o"][:, cc].T
            return_preds[b, sl] = res[r]["rpT_o"][:, cc].T
    return (state_preds, action_preds, return_preds)


# revision 23
# speedup vs baseline: 1.0863x; 1.0216x over previous
"""DecisionTransformer Trainium2 kernel (self-contained).

Sharding: 8 cores = 4 batch pairs x 2 sequence halves. Core r: batch r//2,
half r%2. Half 0 owns timestep blocks {0,3} (of 4 x 128-ts blocks), half 1
owns {1,2} -- balances causal-attention work exactly. Per layer each pair
AllGathers its K/V (bf16, one collective); everything else is token-local
with full weights.

Activations live transposed ([H, tokens]) so matmuls chain with zero
transposes. LayerNorm stats via PE ones-matmul column sums. Softmax skips
max-subtraction (scores are small for this model; verified vs reference).
att@V carries a ones-column per head so softmax denominators fall out of
the same matmul. bf16 matmuls, f32 PSUM accumulation, f32 residual stream.
One SPMD program for all cores; the causal-structure difference between
sequence halves is encoded in per-core mask input data.
"""
import numpy as np
import ml_dtypes

import concourse.bacc as bacc
import concourse.bass as bass
import concourse.tile as tile
from concourse import mybir

bf16 = ml_dtypes.bfloat16

B, T = 4, 512
STATE_DIM, ACT_DIM = 17, 18
H, NH, L, MAXT = 1024, 16, 4, 4096
S = 3 * T
D = H // NH
FF = 4 * H
LN_EPS = 1e-5
HP = H // 128
FFP = FF // 128

GCHUNK = [[0, 3], [1, 2]]

f32 = mybir.dt.float32
bf = mybir.dt.bfloat16
i32 = mybir.dt.int32
AF = mybir.ActivationFunctionType


def _strided(ap, start, step, count):
    return bass.AP(tensor=ap.tensor, offset=ap.offset + start,
                   ap=[list(ap.ap[0]), [step, count]])


def build_program(n_cores, own_chunks_list, uniform, n_layers=None,
                  skip_ag=False, skip_attn=False, skip_mlp=False, skip_qkv=False,
                  skip_coll=False):
    SLOTS = len(own_chunks_list)
    N_TS = 128 * SLOTS
    NT = 3 * N_TS
    NKT = 12 if uniform else 3 * (max(own_chunks_list) + 1)
    slot_valid, slot_masked, masked_pairs = [], [], []
    for j, g in enumerate(own_chunks_list):
        if uniform:
            valid = list(range(6)) if j == 0 else list(range(12))
            masked = list(range(6)) if j == 0 else list(range(6, 12))
        else:
            valid = list(range(3 * (g + 1)))
            masked = valid[-3:]
        slot_valid.append(valid)
        mm_ = {}
        for t in masked:
            mm_[t] = len(masked_pairs)
            masked_pairs.append((j, t))
        slot_masked.append(mm_)
    NMASK = len(masked_pairs)

    nc = bacc.Bacc("TRN2", target_bir_lowering=False, debug=False,
                   num_devices=n_cores)

    ts_idx = nc.dram_tensor("ts_idx", [128, SLOTS], i32, kind="ExternalInput")
    act_idx = nc.dram_tensor("act_idx", [128, SLOTS], i32, kind="ExternalInput")
    states_T = nc.dram_tensor("states_T", [STATE_DIM + 1, N_TS], f32, kind="ExternalInput")
    rtg_T = nc.dram_tensor("rtg_T", [2, N_TS], f32, kind="ExternalInput")
    masks_in = nc.dram_tensor("masks_in", [128, NMASK, 384], bf, kind="ExternalInput")
    ttab = nc.dram_tensor("ttab", [MAXT, H], f32, kind="ExternalInput")
    atab = nc.dram_tensor("atab", [ACT_DIM, H], f32, kind="ExternalInput")
    wst_in = nc.dram_tensor("wst_in", [STATE_DIM + 1, H], f32, kind="ExternalInput")
    wrtg_in = nc.dram_tensor("wrtg_in", [2, H], f32, kind="ExternalInput")
    ident_in = nc.dram_tensor("ident_in", [128, 128], f32, kind="ExternalInput")
    watt = nc.dram_tensor("watt", [L, 4, H, H], bf, kind="ExternalInput")
    w1_in = nc.dram_tensor("w1_in", [L, H, FF], bf, kind="ExternalInput")
    w2_in = nc.dram_tensor("w2_in", [L, FF, H], bf, kind="ExternalInput")
    ba_in = nc.dram_tensor("ba_in", [128, L * 4, HP], f32, kind="ExternalInput")
    b1_in = nc.dram_tensor("b1_in", [128, L, FFP], f32, kind="ExternalInput")
    b2_in = nc.dram_tensor("b2_in", [128, L, HP], f32, kind="ExternalInput")
    bv_bc = nc.dram_tensor("bv_bc", [L, 2, 512], f32, kind="ExternalInput")
    wh_in = nc.dram_tensor("wh_in", [H, 36], bf, kind="ExternalInput")
    bh_in = nc.dram_tensor("bh_in", [36], f32, kind="ExternalInput")

    spT_o = nc.dram_tensor("spT_o", [STATE_DIM, N_TS], f32, kind="ExternalOutput")
    apT_o = nc.dram_tensor("apT_o", [ACT_DIM, N_TS], f32, kind="ExternalOutput")
    rpT_o = nc.dram_tensor("rpT_o", [1, N_TS], f32, kind="ExternalOutput")

    KV_N = H * NT
    if uniform:
        cc_in_t = nc.dram_tensor("cc_in", [2 * KV_N], bf)
        cc_out_t = nc.dram_tensor("cc_out", [4 * KV_N], bf)

    NL = L if n_layers is None else n_layers
    with tile.TileContext(nc) as tc:
        with tc.tile_pool(name="sb", bufs=1) as sbp, \
             tc.tile_pool(name="rot", bufs=3) as rot, \
             tc.tile_pool(name="pp", bufs=8, space="PSUM") as pp:

            def P():
                return pp.tile([128, 512], f32, tag="mm", name="psmm")

            hT = sbp.tile([128, HP, NT], f32, tag="hT")
            h_bf = sbp.tile([128, HP, NT], bf, tag="h_bf")
            oT = sbp.tile([128, HP, NT], bf, tag="oT")
            qT = sbp.tile([128, HP, NT], bf, tag="qT")
            kT_all = sbp.tile([128, HP, 128 * NKT], bf, tag="kT_all")
            v_all = sbp.tile([128, NKT, 16 * 65], bf, tag="v_all")
            masks = sbp.tile([128, NMASK, 384], bf, tag="masks")
            ba_t = sbp.tile([128, L * 4, HP], f32, tag="ba_t")
            b1_t = sbp.tile([128, L, FFP], f32, tag="b1_t")
            b2_t = sbp.tile([128, L, HP], f32, tag="b2_t")
            bh_t = sbp.tile([36, 1], f32, tag="bh_t")
            ones_f = sbp.tile([128, 1], f32, tag="ones_f")
            ones_r = sbp.tile([1, 128], f32, tag="ones_r")
            ones_b = sbp.tile([128, 1], bf, tag="ones_b")
            eps_t = sbp.tile([1, 1], f32, tag="eps_t")
            wh_t = sbp.tile([128, HP, 36], bf, tag="wh_t")

            nc.gpsimd.dma_start(masks[:], masks_in[:])
            nc.gpsimd.dma_start(ba_t[:], ba_in[:])
            nc.gpsimd.dma_start(b1_t[:], b1_in[:])
            nc.gpsimd.dma_start(b2_t[:], b2_in[:])
            nc.gpsimd.dma_start(bh_t[:], bass.AP(tensor=bh_in, offset=0, ap=[[1, 36], [0, 1]]))
            nc.gpsimd.dma_start(wh_t[:], wh_in[:].rearrange("(kt p) o -> p kt o", p=128))
            nc.vector.memset(ones_f[:], 1.0)
            nc.vector.memset(ones_r[:], 1.0)
            nc.vector.memset(ones_b[:], 1.0)
            nc.vector.memset(eps_t[:], LN_EPS)
            for h in range(16):
                nc.vector.memset(v_all[:, :, 65 * h + 64], 1.0)

            # ---------- P0: embeddings ----------
            with tc.tile_pool(name="ep", bufs=1) as ep:
                ident = ep.tile([128, 128], f32, tag="ident")
                sb_ts = ep.tile([128, SLOTS], i32, tag="sb_ts")
                sb_ai = ep.tile([128, SLOTS], i32, tag="sb_ai")
                sb_st = ep.tile([STATE_DIM + 1, N_TS], f32, tag="sb_st")
                sb_rt = ep.tile([2, N_TS], f32, tag="sb_rt")
                wst_t = ep.tile([STATE_DIM + 1, H], f32, tag="wst_t")
                wrtg_t = ep.tile([2, H], f32, tag="wrtg_t")
                nc.gpsimd.dma_start(ident[:], ident_in[:])
                nc.gpsimd.dma_start(sb_ts[:], ts_idx[:])
                nc.gpsimd.dma_start(sb_ai[:], act_idx[:])
                nc.gpsimd.dma_start(sb_st[:], states_T[:])
                nc.gpsimd.dma_start(sb_rt[:], rtg_T[:])
                nc.gpsimd.dma_start(wst_t[:], wst_in[:])
                nc.gpsimd.dma_start(wrtg_t[:], wrtg_in[:])

                for u2 in range(SLOTS):
                    te = ep.tile([128, H], f32, tag="te", bufs=1)
                    nc.gpsimd.indirect_dma_start(
                        out=te[:], out_offset=None, in_=ttab[:],
                        in_offset=bass.IndirectOffsetOnAxis(ap=sb_ts[:, u2:u2 + 1], axis=0))
                    ae = ep.tile([128, H], f32, tag="ae", bufs=1)
                    nc.gpsimd.indirect_dma_start(
                        out=ae[:], out_offset=None, in_=atab[:],
                        in_offset=bass.IndirectOffsetOnAxis(ap=sb_ai[:, u2:u2 + 1], axis=0))
                    for ch in range(2):
                        cs = slice(512 * ch, 512 * ch + 512)
                        nat = [ep.tile([128, 512], f32, tag=f"nat{ty}",
                                       name=f"nat{ty}", bufs=2) for ty in range(3)]
                        nc.vector.tensor_add(nat[2][:], ae[:, cs], te[:, cs])
                        ps = P()
                        nc.tensor.matmul(ps[:], sb_st[:, 128 * u2:128 * u2 + 128],
                                         wst_t[:, cs], start=True, stop=True)
                        nc.vector.tensor_add(nat[1][:], ps[:], te[:, cs])
                        pr_ = P()
                        nc.tensor.matmul(pr_[:], sb_rt[:, 128 * u2:128 * u2 + 128],
                                         wrtg_t[:, cs], start=True, stop=True)
                        nc.vector.tensor_add(nat[0][:], pr_[:], te[:, cs])
                        for ty in range(3):
                            for k4 in range(4):
                                kt = 4 * ch + k4
                                pt = P()
                                nc.tensor.transpose(pt[:, 0:128],
                                                    nat[ty][:, 128 * k4:128 * k4 + 128],
                                                    ident[:])
                                dst = _strided(hT[:, kt, :], 384 * u2 + ty, 3, 128)
                                nc.vector.tensor_copy(dst, pt[:, 0:128])

            # ---------- LN (in-place on hT, refresh h_bf) ----------
            def ln_inplace():
                for c in range(SLOTS):
                    cs = slice(384 * c, 384 * c + 384)
                    ab_t = rot.tile([1, 2, 384], f32, tag="ab_t", name="ab_t", bufs=3)
                    ps1 = P()
                    ps2 = P()
                    for kt in range(HP):
                        sq = rot.tile([128, 384], bf, tag="sq", name="sq", bufs=2)
                        nc.scalar.activation(sq[:], hT[:, kt, cs], AF.Square)
                        nc.tensor.matmul(ps1[0:1, 0:384], ones_f[:], hT[:, kt, cs],
                                         start=(kt == 0), stop=(kt == HP - 1))
                        nc.tensor.matmul(ps2[0:1, 0:384], ones_b[:], sq[:],
                                         start=(kt == 0), stop=(kt == HP - 1))
                    mean = rot.tile([1, 384], f32, tag="mean")
                    e2 = rot.tile([1, 384], f32, tag="e2")
                    nc.scalar.activation(mean[:], ps1[0:1, 0:384], AF.Copy, scale=1.0 / H)
                    nc.scalar.activation(e2[:], ps2[0:1, 0:384], AF.Copy, scale=1.0 / H)
                    var = rot.tile([1, 384], f32, tag="var")
                    nc.vector.tensor_mul(var[:], mean[:], mean[:])
                    nc.vector.tensor_tensor(out=var[:], in0=e2[:], in1=var[:],
                                            op=mybir.AluOpType.subtract)
                    nc.scalar.activation(var[:], var[:], AF.Sqrt, bias=eps_t[:], scale=1.0)
                    nc.vector.reciprocal(ab_t[:, 0, :], var[:])
                    nc.vector.tensor_mul(ab_t[:, 1, :], mean[:], ab_t[:, 0, :])
                    nc.scalar.mul(ab_t[:, 1, :], ab_t[:, 1, :], -1.0)
                    pa = P()
                    nc.tensor.matmul(pa[:, 0:384], ones_r[:], ab_t[:, 0, :],
                                     start=True, stop=True)
                    pb = P()
                    nc.tensor.matmul(pb[:, 0:384], ones_r[:], ab_t[:, 1, :],
                                     start=True, stop=True)
                    for kt in range(HP):
                        nc.vector.tensor_mul(hT[:, kt, cs], hT[:, kt, cs], pa[:, 0:384])
                        nc.vector.tensor_add(hT[:, kt, cs], hT[:, kt, cs], pb[:, 0:384])
                        nc.vector.tensor_copy(h_bf[:, kt, cs], hT[:, kt, cs])

            ln_inplace()

            # ---------- layers ----------
            with tc.tile_pool(name="wp", bufs=4) as wp:

                def load_half(src_ap, n_mid):
                    w = wp.tile([128, n_mid, 8 * 1024 // n_mid // 2], bf,
                                tag="w", name="wtile")
                    nc.gpsimd.dma_start(w[:], src_ap)
                    return w

                def load_att_w(l, j, half):
                    # [128, HP, 512]: out-cols half*512 ... half*512+512
                    return load_half(
                        watt[l, j][:, 512 * half:512 * half + 512].rearrange(
                            "(kt p) o -> p kt o", p=128), HP)

                for l_iter in range(NL):
                    l = l_iter % L
                    # ---- QKV ----
                    for (is_q, bj) in (() if skip_qkv else ((True, 0), (False, 1))):
                        for half in range(2):
                            wt = load_att_w(l, bj, half)
                            for o4 in range(4):
                                ot = 4 * half + o4
                                for c in range(SLOTS):
                                    cs = slice(384 * c, 384 * c + 384)
                                    ps = P()
                                    for kt in range(HP):
                                        nc.tensor.matmul(
                                            ps[:, 0:384], wt[:, kt, 128 * o4:128 * o4 + 128],
                                            h_bf[:, kt, cs],
                                            start=(kt == 0), stop=(kt == HP - 1))
                                    if is_q:
                                        nc.scalar.activation(
                                            qT[:, ot, cs], ps[:, 0:384], AF.Identity,
                                            bias=ba_t[:, 4 * l + bj, ot:ot + 1])
                                    else:
                                        g = own_chunks_list[c]
                                        nc.scalar.activation(
                                            kT_all[:, ot, 384 * g:384 * g + 384],
                                            ps[:, 0:384], AF.Identity,
                                            bias=ba_t[:, 4 * l + bj, ot:ot + 1])
                    for dc in (() if skip_qkv else range(2)):
                        wv = load_att_w(l, 2, dc)
                        bvb = rot.tile([128, 512], f32, tag="tmp", name="bvb", bufs=1)
                        nc.gpsimd.dma_start(
                            bvb[:], bass.AP(tensor=bv_bc, offset=(l * 2 + dc) * 512,
                                            ap=[[0, 128], [1, 512]]))
                        for tt in range(NT // 128):
                            g = own_chunks_list[tt // 3]
                            gt = 3 * g + (tt % 3)
                            ps = P()
                            for kt in range(HP):
                                nc.tensor.matmul(ps[:], h_bf[:, kt, 128 * tt:128 * tt + 128],
                                                 wv[:, kt, :],
                                                 start=(kt == 0), stop=(kt == HP - 1))
                            dst = bass.AP(
                                tensor=v_all.tensor,
                                offset=v_all[:].offset + gt * (16 * 65) + (8 * dc) * 65,
                                ap=[list(v_all[:].ap[0]), [65, 8], [1, 64]])
                            nc.vector.tensor_tensor(
                                out=dst, in0=ps[:].rearrange("p (h d) -> p h d", h=8),
                                in1=bvb[:].rearrange("p (h d) -> p h d", h=8),
                                op=mybir.AluOpType.add)

                    # ---- AllGather K/V ----
                    if uniform and not skip_ag:
                        cc_in = cc_in_t.ap()
                        cc_out = cc_out_t.ap()
                        k_sec = cc_in[0:KV_N].rearrange("(r c) -> r c", c=NT)
                        v_sec = cc_in[KV_N:2 * KV_N].rearrange("(r c) -> r c", c=H)
                        for c in range(SLOTS):
                            g = own_chunks_list[c]
                            nc.gpsimd.dma_start(
                                k_sec[:, 384 * c:384 * c + 384].rearrange(
                                    "(kt p) q -> p kt q", p=128),
                                kT_all[:, :, 384 * g:384 * g + 384])
                            for tt in range(3):
                                src = bass.AP(
                                    tensor=v_all.tensor,
                                    offset=v_all[:].offset + (3 * g + tt) * (16 * 65),
                                    ap=[list(v_all[:].ap[0]), [65, 16], [1, 64]])
                                nc.sync.dma_start(
                                    v_sec[384 * c + 128 * tt:384 * c + 128 * tt + 128,
                                          :].rearrange("p (h d) -> p h d", h=16), src)
                        if not skip_coll:
                            groups = [[2 * i, 2 * i + 1] for i in range(n_cores // 2)]
                            nc.gpsimd.collective_compute(
                                "AllGather", mybir.AluOpType.bypass, replica_groups=groups,
                                ins=[cc_in], outs=[cc_out])
                        else:
                            nc.sync.dma_start(cc_out[0:2 * KV_N], cc_in[:])
                        for rr in range(2):
                            sec = cc_out[rr * 2 * KV_N: (rr + 1) * 2 * KV_N]
                            k_s = sec[0:KV_N].rearrange("(r c) -> r c", c=NT)
                            v_s = sec[KV_N:2 * KV_N].rearrange("(r c) -> r c", c=H)
                            for c in range(SLOTS):
                                g = GCHUNK[rr][c]
                                nc.gpsimd.dma_start(
                                    kT_all[:, :, 384 * g:384 * g + 384],
                                    k_s[:, 384 * c:384 * c + 384].rearrange(
                                        "(kt p) q -> p kt q", p=128))
                                for tt in range(3):
                                    dstv = bass.AP(
                                        tensor=v_all.tensor,
                                        offset=v_all[:].offset + (3 * g + tt) * (16 * 65),
                                        ap=[list(v_all[:].ap[0]), [65, 16], [1, 64]])
                                    nc.sync.dma_start(
                                        dstv,
                                        v_s[384 * c + 128 * tt:384 * c + 128 * tt + 128,
                                            :].rearrange("p (h d) -> p h d", h=16))

                    # ---- attention ----
                    if skip_attn:
                        nc.vector.memset(oT[:], 0.0)
                    for hp_ in (() if skip_attn else range(8)):
                        pos = {j: [P(), P()] for j in range(SLOTS)}
                        nmax = max(len(slot_valid[j]) for j in range(SLOTS))
                        for ti in range(nmax):
                            for j in range(SLOTS):
                                valid = slot_valid[j]
                                if ti >= len(valid):
                                    continue
                                t = valid[ti]
                                qs = slice(384 * j, 384 * j + 384)
                                for hh in range(2):
                                    h = 2 * hp_ + hh
                                    bp = 64 * hh
                                    ps = P()
                                    nc.tensor.matmul(
                                        ps[:, 0:384],
                                        kT_all[bp:bp + 64, hp_, 128 * t:128 * t + 128],
                                        qT[bp:bp + 64, hp_, qs], start=True, stop=True)
                                    es = rot.tile([128, 384], bf, tag="es", name="es", bufs=4)
                                    nc.scalar.activation(es[:], ps[:, 0:384], AF.Exp,
                                                         scale=0.125)
                                    mi = slot_masked[j].get(t)
                                    if mi is not None:
                                        nc.vector.tensor_mul(es[:], es[:], masks[:, mi, :])
                                    nc.tensor.matmul(
                                        pos[j][hh][0:65, 0:384],
                                        v_all[:, t, 65 * h:65 * h + 65], es[:],
                                        start=(ti == 0), stop=(ti == len(valid) - 1))
                        for j in (() if skip_attn else range(SLOTS)):
                            qs = slice(384 * j, 384 * j + 384)
                            po = pos[j]
                            rcp = rot.tile([1, 2, 384], f32, tag="rcp", name="rcp", bufs=4)
                            for hh in range(2):
                                nc.vector.reciprocal(rcp[:, hh, :], po[hh][64:65, 0:384])
                            for hh in range(2):
                                pr = P()
                                nc.tensor.matmul(pr[0:64, 0:384], ones_r[:, 0:64],
                                                 rcp[:, hh, :], start=True, stop=True)
                                osl = oT[64 * hh:64 * hh + 64, hp_, qs]
                                nc.vector.tensor_copy(osl, po[hh][0:64, 0:384])
                                nc.vector.tensor_mul(osl, osl, pr[0:64, 0:384])

                    # ---- proj + residual + LN ----
                    for half in range(2):
                        wo = load_att_w(l, 3, half)
                        for o4 in range(4):
                            ot = 4 * half + o4
                            for c in range(SLOTS):
                                cs = slice(384 * c, 384 * c + 384)
                                ps = P()
                                for kt in range(HP):
                                    nc.tensor.matmul(
                                        ps[:, 0:384], wo[:, kt, 128 * o4:128 * o4 + 128],
                                        oT[:, kt, cs], start=(kt == 0), stop=(kt == HP - 1))
                                tmp = rot.tile([128, 512], f32, tag="tmp", name="tmp", bufs=1)
                                nc.scalar.activation(tmp[:, 0:384], ps[:, 0:384], AF.Identity,
                                                     bias=ba_t[:, 4 * l + 3, ot:ot + 1])
                                nc.vector.tensor_add(hT[:, ot, cs], hT[:, ot, cs],
                                                     tmp[:, 0:384])
                    ln_inplace()

                    # ---- MLP (accumulate groups directly into hT) ----
                    for grp in (() if skip_mlp else range(4)):
                        w1a = load_half(
                            w1_in[l][:, 1024 * grp:1024 * grp + 512].rearrange(
                                "(kt p) o -> p kt o", p=128), HP)
                        w1b = load_half(
                            w1_in[l][:, 1024 * grp + 512:1024 * grp + 1024].rearrange(
                                "(kt p) o -> p kt o", p=128), HP)
                        w2a = load_half(
                            w2_in[l][1024 * grp:1024 * grp + 512, :].rearrange(
                                "(ft p) o -> p ft o", p=128), 4)
                        w2b = load_half(
                            w2_in[l][1024 * grp + 512:1024 * grp + 1024, :].rearrange(
                                "(ft p) o -> p ft o", p=128), 4)
                        for c in range(NT // 384):
                            cs = slice(384 * c, 384 * c + 384)
                            mst = rot.tile([128, 8, 384], bf, tag="mst", name="mst", bufs=2)
                            for ft in range(8):
                                w1h = w1a if ft < 4 else w1b
                                f4 = ft % 4
                                pm = P()
                                for kt in range(HP):
                                    nc.tensor.matmul(
                                        pm[:, 0:384], w1h[:, kt, 128 * f4:128 * f4 + 128],
                                        h_bf[:, kt, cs],
                                        start=(kt == 0), stop=(kt == HP - 1))
                                nc.scalar.activation(
                                    mst[:, ft, :], pm[:, 0:384], AF.Gelu,
                                    bias=b1_t[:, l, 8 * grp + ft:8 * grp + ft + 1])
                            for oh in range(2):
                                ph = [P() for _ in range(4)]
                                for ft in range(8):
                                    w2h = w2a if ft < 4 else w2b
                                    f4 = ft % 4
                                    for o4 in range(4):
                                        ot = 4 * oh + o4
                                        nc.tensor.matmul(
                                            ph[o4][:, 0:384],
                                            w2h[:, f4, 128 * ot:128 * ot + 128],
                                            mst[:, ft, :], start=(ft == 0), stop=(ft == 7))
                                for o4 in range(4):
                                    ot = 4 * oh + o4
                                    nc.vector.tensor_add(hT[:, ot, cs], hT[:, ot, cs],
                                                         ph[o4][:, 0:384])
                    # b2 bias
                    for ot in range(HP):
                        nc.vector.tensor_scalar(
                            out=hT[:, ot, :], in0=hT[:, ot, :],
                            scalar1=b2_t[:, l, ot:ot + 1], scalar2=None,
                            op0=mybir.AluOpType.add)
                    ln_inplace()

                # ---------- final heads ----------
                for c in range(SLOTS):
                    ps = P()
                    for kt in range(HP):
                        rhs_a = _strided(h_bf[:, kt, :], 384 * c + 2, 3, 128)
                        nc.tensor.matmul(ps[0:36, 0:128], wh_t[:, kt, :], rhs_a,
                                         start=(kt == 0), stop=(kt == HP - 1))
                    out_s = rot.tile([36, 128], f32, tag="out_s", name="out_s")
                    nc.scalar.activation(out_s[:], ps[0:36, 0:128], AF.Identity, bias=bh_t[:])
                    nc.gpsimd.dma_start(spT_o[:, 128 * c:128 * c + 128], out_s[0:17, :])
                    nc.gpsimd.dma_start(rpT_o[:, 128 * c:128 * c + 128], out_s[35:36, :])
                    ps2 = P()
                    for kt in range(HP):
                        rhs_s = _strided(h_bf[:, kt, :], 384 * c + 1, 3, 128)
                        nc.tensor.matmul(ps2[0:36, 0:128], wh_t[:, kt, :], rhs_s,
                                         start=(kt == 0), stop=(kt == HP - 1))
                    out_s2 = rot.tile([36, 128], f32, tag="out_s", name="out_s2")
                    nc.scalar.activation(out_s2[:], ps2[0:36, 0:128], AF.Identity,
                                         bias=bh_t[:])
                    nc.gpsimd.dma_start(apT_o[:, 128 * c:128 * c + 128], out_s2[17:35, :])

    nc.compile()
    meta = dict(SLOTS=SLOTS, N_TS=N_TS, NT=NT, NKT=NKT,
                masked_pairs=masked_pairs, own_chunks_list=own_chunks_list)
    return nc, meta


# ====================== host side ======================

def make_masks(meta):
    oc = meta["own_chunks_list"]
    NM = len(meta["masked_pairs"])
    m = np.zeros((128, NM, 384), np.float32)
    for i, (j, t) in enumerate(meta["masked_pairs"]):
        g = oc[j]
        kpos = 128 * t + np.arange(128)[:, None]
        qpos = 384 * g + np.arange(384)[None, :]
        m[:, i, :] = (kpos <= qpos).astype(np.float32)
    return m.astype(bf16)


def make_core_inputs(inputs, half, b, meta, shared):
    chunks = meta["own_chunks_list"]
    ts_sl = np.concatenate([np.arange(128 * g, 128 * g + 128) for g in chunks])
    d = dict(shared)
    d["ts_idx"] = np.asarray(inputs["timesteps"])[b, ts_sl].astype(np.int32).reshape(
        len(chunks), 128).T.copy()
    d["act_idx"] = np.asarray(inputs["actions"])[b, ts_sl].astype(np.int32).reshape(
        len(chunks), 128).T.copy()
    st = np.asarray(inputs["states"], np.float32)[b, ts_sl].T  # [17, N_TS]
    d["states_T"] = np.concatenate([st, np.ones((1, st.shape[1]), np.float32)])
    rt = np.asarray(inputs["returns_to_go"], np.float32)[b, ts_sl].T  # [1, N_TS]
    d["rtg_T"] = np.concatenate([rt, np.ones((1, rt.shape[1]), np.float32)])
    return d


class SpmdRunner:
    def __init__(self, nc, n_cores):
        import jax
        from jax.sharding import Mesh, PartitionSpec
        from jax.experimental.shard_map import shard_map
        from concourse.bass2jax import (_bass_exec_p, install_neuronx_cc_hook,
                                        partition_id_tensor)
        import concourse.mybir as mybir_
        self.jax = jax
        install_neuronx_cc_hook()
        self.nc = nc
        self.n_cores = n_cores
        partition_name = nc.partition_id_tensor.name if nc.partition_id_tensor else None
        in_names, out_names, out_avals = [], [], []
        for alloc in nc.m.functions[0].allocations:
            if not isinstance(alloc, mybir_.MemoryLocationSet):
                continue
            name = alloc.memorylocations[0].name
            if alloc.kind == "ExternalInput":
                if name != partition_name:
                    in_names.append(name)
            elif alloc.kind == "ExternalOutput":
                out_names.append(name)
                out_avals.append(jax.core.ShapedArray(
                    tuple(alloc.tensor_shape), mybir_.dt.np(alloc.dtype)))
        self.in_names, self.out_names, self.out_avals = in_names, out_names, out_avals
        n_params = len(in_names)
        self._zero_outs = [np.zeros(a.shape, a.dtype) for a in out_avals]
        all_in = list(in_names) + list(out_names)
        if partition_name is not None:
            all_in.append(partition_name)

        def _body(*args):
            operands = list(args)
            if partition_name is not None:
                operands.append(partition_id_tensor())
            outs = _bass_exec_p.bind(
                *operands, out_avals=tuple(out_avals), in_names=tuple(all_in),
                out_names=tuple(out_names), lowering_input_output_aliases=(),
                sim_require_finite=False, sim_require_nnan=False, nc=nc)
            return tuple(outs)

        devices = jax.devices()[:n_cores]
        self.mesh = Mesh(np.asarray(devices), ("core",))
        in_specs = (PartitionSpec("core"),) * (n_params + len(out_names))
        out_specs = (PartitionSpec("core"),) * len(out_names)
        self._fn = jax.jit(shard_map(_body, mesh=self.mesh, in_specs=in_specs,
                                     out_specs=out_specs, check_rep=False))

    def stage_inputs(self, in_maps):
        import jax
        from jax.sharding import PartitionSpec
        n = self.n_cores
        per_core = [[np.asarray(m[name]) for name in self.in_names] for m in in_maps]
        concat = [np.concatenate([per_core[c][i] for c in range(n)], axis=0)
                  for i in range(len(self.in_names))]
        concat += [np.zeros((n * z.shape[0], *z.shape[1:]), z.dtype)
                   for z in self._zero_outs]
        sharding = jax.sharding.NamedSharding(self.mesh, PartitionSpec("core"))
        return [jax.device_put(c, sharding) for c in concat]

    def run_staged(self, staged):
        out = self._fn(*staged)
        self.jax.block_until_ready(out)
        return out

    def results(self, out_arrs):
        n = self.n_cores
        return [{name: np.asarray(out_arrs[i]).reshape(n, *self.out_avals[i].shape)[c]
                 for i, name in enumerate(self.out_names)} for c in range(n)]

    def run(self, in_maps):
        return self.results(self.run_staged(self.stage_inputs(in_maps)))


def _shared_inputs(inputs, meta):
    sh = {}
    sh["masks_in"] = make_masks(meta)
    sh["ttab"] = np.asarray(inputs["embed_time_table"], np.float32)
    sh["atab"] = np.asarray(inputs["embed_act_table"], np.float32)
    sh["wst_in"] = np.concatenate([np.asarray(inputs["Wst"], np.float32),
                                   np.asarray(inputs["bst"], np.float32)[None, :]])
    sh["wrtg_in"] = np.concatenate([np.asarray(inputs["Wrtg"], np.float32),
                                    np.asarray(inputs["brtg"], np.float32)[None, :]])
    sh["ident_in"] = np.eye(128, dtype=np.float32)
    watt = np.stack([np.asarray(inputs[k], np.float32) for k in
                     ("Wq", "Wk", "Wv", "Wo")], axis=1)
    sh["watt"] = watt.astype(bf16)
    sh["w1_in"] = np.asarray(inputs["W1"], np.float32).astype(bf16)
    sh["w2_in"] = np.asarray(inputs["W2"], np.float32).astype(bf16)
    ba = np.stack([np.asarray(inputs[k], np.float32) for k in
                   ("bq", "bk", "bv", "bo")], axis=1)          # [L,4,H]
    sh["ba_in"] = np.ascontiguousarray(
        ba.reshape(L, 4, HP, 128).transpose(3, 0, 1, 2).reshape(128, L * 4, HP))
    b1 = np.asarray(inputs["b1"], np.float32)                   # [L,FF]
    sh["b1_in"] = np.ascontiguousarray(
        b1.reshape(L, FFP, 128).transpose(2, 0, 1))
    b2 = np.asarray(inputs["b2"], np.float32)                   # [L,H]
    sh["b2_in"] = np.ascontiguousarray(
        b2.reshape(L, HP, 128).transpose(2, 0, 1))
    sh["bv_bc"] = np.ascontiguousarray(ba[:, 2, :].reshape(L, 2, 512))
    wh = np.concatenate([np.asarray(inputs["Wps"], np.float32),
                         np.asarray(inputs["Wpa"], np.float32),
                         np.asarray(inputs["Wpr"], np.float32)], axis=1)
    sh["wh_in"] = wh.astype(bf16)
    sh["bh_in"] = np.concatenate([np.asarray(inputs["bps"], np.float32),
                                  np.asarray(inputs["bpa"], np.float32),
                                  np.asarray(inputs["bpr"], np.float32)])
    return sh


_CACHE = {}


def kernel(**inputs):
    if "runner" not in _CACHE:
        nc, meta0 = build_program(8, GCHUNK[0], uniform=True)
        _CACHE["prog"] = (nc, meta0)
        _CACHE["runner"] = SpmdRunner(nc, 8)
        _CACHE["meta_by_half"] = [dict(meta0, own_chunks_list=GCHUNK[0]),
                                  dict(meta0, own_chunks_list=GCHUNK[1])]
    runner = _CACHE["runner"]
    meta_by_half = _CACHE["meta_by_half"]
    shared = [_shared_inputs(inputs, meta_by_half[0])]
    sh1 = dict(shared[0])
    sh1["masks_in"] = make_masks(meta_by_half[1])
    shared.append(sh1)
    in_maps = [make_core_inputs(inputs, r % 2, r // 2, meta_by_half[r % 2],
                                shared[r % 2]) for r in range(8)]
    staged = runner.stage_inputs(in_maps)
    _CACHE["last_staged"] = staged
    res = runner.results(runner.run_staged(staged))

    state_preds = np.zeros((B, T, STATE_DIM), np.float32)
    action_preds = np.zeros((B, T, ACT_DIM), np.float32)
    return_preds = np.zeros((B, T, 1), np.float32)
    for r in range(8):
        b, half = r // 2, r % 2
        for c, g in enumerate(GCHUNK[half]):
            sl = slice(128 * g, 128 * g + 128)
            cc = slice(128 * c, 128 * c + 128)
            state_preds[b, sl] = res[r]["spT_o"][:, cc].T
            action_preds[b, sl] = res[r]["apT_o"][:, cc].T
            return_preds[b, sl] = res[r]["rpT_o"][:, cc].T
    return (state_preds, action_preds, return_preds)
